# revision 2
# baseline (speedup 1.0000x reference)
"""Trainium2 Bass kernel for sparse-attention 3D-ViT (nn_BaseModel_44341242364529).

No-collective design: the layer-1 AllGather of band k/v is replaced by
redundant layer-0 compute.  Each core computes layer 0 (attention + FFN)
for its full 12-tile query band (rows c*512-1024 .. c*512+512), the two
corner tiles EOS attends (29, 31), and the BOS/EOS tile -- so layer-1
k/v for every key any own-query needs is produced locally and no core
ever waits on another (cross-core barrier waits dominated the measured
exec time of the collective version).

Mechanics (inherited from the collective baseline):
- all matmuls bf16, fp32 PSUM accumulation
- multiplicative attention bias: P = exp(scale*S) * expW, expW tables
  host-precomputed (masked slots = 0; fully-padded query rows attend BOS
  with weight 1 so their denominator stays finite); tables are STREAMED
  from DRAM into the P tile and multiplied in place (16MB total, too big
  to keep resident)
- attention runs in 4-query-tile "group passes" over a 12-key-tile
  window (the RUNS table): layer 0 = 3 band passes, layer 1 = 1 pass;
  corner/BOS-EOS queries use small dense per-tile passes
- w-grouped S matmuls (stationary kT tile), transposed AV (stationary
  65-col v tile with ones column for the denominator), per-head wo with
  the softmax denominator folded into the residual update
- LN per-tile (bn_stats -> sqrt+recip -> gpsimd apply -> DMA transposes)
"""

import numpy as np
import ml_dtypes

# model dims (hardcoded per spec)
IMG, PATCH, D, H, NLAYERS, DFF = 64, 4, 256, 4, 2, 1024
GT = IMG // PATCH          # 16
N = GT * GT * GT           # 4096
L = N + 2                  # 4098
DH = D // H                # 64
PVOL = PATCH ** 3          # 64
NCORES = 8
LC = 512                   # real patch rows per core
LLOC = 640                 # padded local rows (5 tiles of 128)
SCALE = 1.0 / np.sqrt(DH)  # 0.125
BF16 = ml_dtypes.bfloat16

NBK = 20                   # band key tiles (global tiles 4c-16 .. 4c+3)
NBQ = 12                   # band query tiles (= band key tiles 8..19)
CKT = [21, 23, 24, 25, 26, 27, 28, 29, 30, 31]   # corner key tiles (global)
NCK = len(CKT)
# corner query tiles 29, 31: key lists in corner-array indices (+ BOS slot)
KTS29 = [0, 2, 3, 4, 5, 6, 7]
KTS31 = [1, 4, 5, 6, 7, 8, 9]
NQT = 15                   # x_q slots: 12 band + 2 corner + 1 BOS/EOS

# w-grouped attention runs for a 4-query-tile group over a 12-tile key
# window: (w, tlist, av_start, av_stop).  w numeric = window tile, "loc4"
# = the local BOS/EOS tile (BOS key at partition 0).
RUNS = [("loc4", (0, 1, 2, 3), True, False),
        (8, (0, 1, 2, 3), False, False), (9, (1, 2, 3), False, False),
        (10, (2, 3), False, False), (11, (3,), False, False),
        (0, (0,), False, False), (1, (1,), False, False), (2, (2,), False, False),
        (3, (0,), False, False), (3, (3,), False, False),
        (4, (0, 1), False, False), (5, (0, 1, 2), False, False),
        (6, (0, 1, 2, 3), False, False), (7, (0, 1, 2, 3), False, True)]
RUNCOLS = []
_off = 0
for _w, _ts, _a, _b in RUNS:
    RUNCOLS.append(_off)
    _off += len(_ts) * 128
EXPW_COLS = _off  # 4096

# run groups for coarse exp/mult: contiguous run ranges, <=1024 cols each
GROUPS = []
_g0 = 0
for _ri in range(len(RUNS) + 1):
    if (_ri == len(RUNS) or _ri == 5
            or (RUNCOLS[_ri] - RUNCOLS[_g0]) + len(RUNS[_ri][1]) * 128 > 1024):
        GROUPS.append((_g0, _ri, RUNCOLS[_g0],
                       (RUNCOLS[_ri - 1] + len(RUNS[_ri - 1][1]) * 128) - RUNCOLS[_g0]))
        _g0 = _ri
        if _ri == len(RUNS):
            break


_prog_cache = {}


def _build_program():
    import concourse.bass as bass
    import concourse.bacc as bacc
    import concourse.tile as tile
    from concourse import mybir

    f32 = mybir.dt.float32
    bf16 = mybir.dt.bfloat16
    AF = mybir.ActivationFunctionType
    nc = bacc.Bacc("TRN2", target_bir_lowering=False, debug=False,
                   num_devices=NCORES)

    def din(name, shape, dt_=bf16):
        return nc.declare_dram_parameter(name, list(shape), dt_, isOutput=False)

    imgT_d = din("imgT", [PVOL, NBK * 128])
    imgTc_d = din("imgTc", [PVOL, NCK * 128])
    emb4_d = din("emb4", [128, D], f32)
    pw_d = din("patch_w", [PVOL, D])
    wq_d = din("wq", [NLAYERS, D, D])
    wk_d = din("wk", [NLAYERS, D, D])
    wv_d = din("wv", [NLAYERS, D, D])
    wo_d = din("wo", [NLAYERS, D, D])
    w1_d = din("w1", [NLAYERS, D, DFF])
    w2_d = din("w2", [NLAYERS, DFF, D])
    expA_d = din("expA", [H, 128, EXPW_COLS])   # band pass A (queries 0..3)
    expB_d = din("expB", [H, 128, EXPW_COLS])   # band pass B (queries 4..7)
    expC_d = din("expC", [H, 128, EXPW_COLS])   # band pass C / layer-1 pass
    expCr_d = din("expCr", [2, H, 128, 8 * 128])  # corner queries 29, 31
    expB4_d = din("expB4", [H, 128, 3 * 128])     # BOS/EOS/pad queries
    ident_d = din("ident", [128, 128])            # PE-transpose identity
    out_d = nc.declare_dram_parameter("out", [LLOC, D], f32, isOutput=True)

    from contextlib import ExitStack
    with tile.TileContext(nc) as tc, ExitStack() as ctx:
        sing = ctx.enter_context(tc.tile_pool(name="sing", bufs=1))
        wk_pool = ctx.enter_context(tc.tile_pool(name="wrk", bufs=1))
        wk2_pool = ctx.enter_context(tc.tile_pool(name="wrk2", bufs=2))
        tmp_pool = ctx.enter_context(tc.tile_pool(name="tmp", bufs=8))
        pe_pool = ctx.enter_context(tc.tile_pool(name="pexp", bufs=4))
        pm_pool = ctx.enter_context(tc.tile_pool(name="pmul", bufs=4))
        ps_s = ctx.enter_context(tc.tile_pool(name="pss", bufs=2, space="PSUM"))
        ps_aoT = ctx.enter_context(tc.tile_pool(name="pst", bufs=2, space="PSUM"))
        ps_pr = ctx.enter_context(tc.tile_pool(name="psp", bufs=2, space="PSUM"))

        sync = nc.sync

        # ---------------- constants / weights ----------------
        # DMA queue drains serially: request in first-use order
        pw = sing.tile([PVOL, D], bf16, tag="pw")
        sync.dma_start(out=pw[:], in_=pw_d[:, :])
        ident = sing.tile([128, 128], bf16, tag="ident")
        sync.dma_start(out=ident[:], in_=ident_d[:, :])
        imgT = sing.tile([PVOL, NBK * 128], bf16, tag="imgT")
        sync.dma_start(out=imgT[:], in_=imgT_d[:, :])
        imgTc = sing.tile([PVOL, NCK * 128], bf16, tag="imgTc")
        sync.dma_start(out=imgTc[:], in_=imgTc_d[:, :])
        emb4 = sing.tile([128, D], f32, tag="emb4")
        sync.dma_start(out=emb4[:], in_=emb4_d[:, :])

        W = {}

        def loadw(nm, dt_, l):
            kd = 8 if nm == "w2" else 2
            t_ = sing.tile([128, kd, dt_.shape[2]], bf16, tag=f"{nm}{l}")
            sync.dma_start(out=t_[:], in_=dt_[l].rearrange("(k p) n -> p k n", p=128))
            W[(nm, l)] = t_

        def loadwoh(l):
            t_ = sing.tile([64, H, D], bf16, tag=f"woh{l}")
            sync.dma_start(out=t_[:], in_=wo_d[l].rearrange("(h p) n -> p h n", p=64))
            W[("woh", l)] = t_

        for nm, dt_ in (("wk", wk_d), ("wv", wv_d), ("wq", wq_d)):
            loadw(nm, dt_, 0)
        loadwoh(0)
        loadw("wo", wo_d, 0)

        def load_rest():
            for nm, dt_ in (("w1", w1_d), ("w2", w2_d)):
                loadw(nm, dt_, 0)
            for nm, dt_ in (("wk", wk_d), ("wv", wv_d), ("wq", wq_d),
                            ("wo", wo_d), ("w1", w1_d), ("w2", w2_d)):
                loadw(nm, dt_, 1)
            loadwoh(1)

        eps_sb = sing.tile([128, 1], f32, tag="eps")
        nc.vector.memset(eps_sb[:], 1e-5)

        # ---------------- persistent activations ----------------
        x_q = wk_pool.tile([128, NQT, D], f32, tag="xq")
        hT = wk_pool.tile([128, 2, 31 * 128], bf16, tag="hT")
        qT0 = wk_pool.tile([128, 2, NQT * 128], bf16, tag="qT0")
        qT1 = wk_pool.tile([128, 2, 5 * 128], bf16, tag="qT1")
        kT_band = wk_pool.tile([128, 2, NBK * 128], bf16, tag="kband")
        v_band = wk_pool.tile([128, NBK, H, DH + 1], bf16, tag="vband")
        kT_cor = wk_pool.tile([128, 2, NCK * 128], bf16, tag="kcor")
        v_cor = wk_pool.tile([128, NCK, H, DH + 1], bf16, tag="vcor")
        kT4 = wk_pool.tile([128, 2, 128], bf16, tag="kT4")
        v4 = wk_pool.tile([128, H, DH + 1], bf16, tag="v4")
        kT1b = wk_pool.tile([128, 2, NBQ * 128], bf16, tag="k1b")
        v1b = wk_pool.tile([128, NBQ, H, DH + 1], bf16, tag="v1b")
        kT1x = wk_pool.tile([128, 2, 256], bf16, tag="k1x")
        v1x = wk_pool.tile([128, 2, H, DH + 1], bf16, tag="v1x")
        kT41 = wk_pool.tile([128, 2, 128], bf16, tag="kT41")
        v41 = wk_pool.tile([128, H, DH + 1], bf16, tag="v41")
        hT2 = wk_pool.tile([128, 2, NQT * 128], bf16, tag="hT2")
        yT = wk_pool.tile([128, 8, 5 * 128], bf16, tag="yT")
        stats = wk_pool.tile([128, 32, 6], f32, tag="stats")
        mv = wk_pool.tile([128, 32, 2], f32, tag="mv")
        rstd = wk_pool.tile([128, 32], f32, tag="rstd")
        o_sb = wk_pool.tile([128, 5, D], f32, tag="osb")
        aoT_sb = wk_pool.tile([128, H, 512], bf16, tag="aoTsb")

        # ones columns for the denominator trick
        nc.vector.memset(v_band[:, :, :, DH:DH + 1], 1.0)
        nc.vector.memset(v_cor[:, :, :, DH:DH + 1], 1.0)
        nc.vector.memset(v4[:, :, DH:DH + 1], 1.0)
        nc.vector.memset(v1b[:, :, :, DH:DH + 1], 1.0)
        nc.vector.memset(v1x[:, :, :, DH:DH + 1], 1.0)
        nc.vector.memset(v41[:, :, DH:DH + 1], 1.0)

        # ---------------- helpers ----------------
        _ln_i = [0]

        def ln_batch(pairs):
            """pairs: [(src f32 [128,D], dst bf16/f32 [128,D])], stage-batched
            (all stats, one sqrt, one recip, all applies) so each engine's
            FIFO streams independent work instead of per-tile chains."""
            n = len(pairs)
            assert n <= 16
            i0 = 0 if _ln_i[0] % 32 + n > 32 else _ln_i[0] % 32
            _ln_i[0] = i0 + n
            for i, (src, dst) in enumerate(pairs):
                nc.vector.bn_stats(out=stats[:, i0 + i, :], in_=src)
                nc.vector.bn_aggr(out=mv[:, i0 + i, :], in_=stats[:, i0 + i, :])
            nc.scalar.activation(out=rstd[:, i0:i0 + n],
                                 in_=mv[:, i0:i0 + n, 1],
                                 func=AF.Sqrt, bias=eps_sb[:], scale=1.0)
            nc.vector.reciprocal(out=rstd[:, i0:i0 + n], in_=rstd[:, i0:i0 + n])
            for i, (src, dst) in enumerate(pairs):
                nc.gpsimd.tensor_scalar(out=dst, in0=src,
                                        scalar1=mv[:, i0 + i, 0:1],
                                        scalar2=rstd[:, i0 + i:i0 + i + 1],
                                        op0=mybir.AluOpType.subtract,
                                        op1=mybir.AluOpType.mult)

        def pe_transpose(srcs, dst, slot0):
            """PE-transpose <=2 row-major bf16 [128,D] tiles into feature-major
            hT slots (keeps the DMA queue free; bf16 PSUM pass-through)."""
            n = len(srcs)
            ps = ps_aoT.tile([128, 1024], bf16, tag="tr", bufs=1)
            with nc.allow_low_precision(reason="transpose is pass-through"):
                for i, src in enumerate(srcs):
                    for j in range(2):
                        nc.tensor.matmul(
                            ps[:, (2 * i + j) * 128:(2 * i + j + 1) * 128],
                            lhsT=src[:, j * 128:(j + 1) * 128], rhs=ident[:],
                            is_transpose=True, start=True, stop=True)
            nc.scalar.copy(
                out=dst[:, :, slot0 * 128:(slot0 + n) * 128].rearrange(
                    "p j (t c) -> p j t c", t=n),
                in_=ps[:, 0:n * 256].rearrange("p (t j c) -> p j t c",
                                               j=2, c=128))

        def k_proj(wsb, hslot, kdst, kcols):
            """kdst[:, j, kcols:+128] = wk_j^T @ h  for j in 0..1"""
            for j in range(2):
                ps = ps_pr.tile([128, 512], f32, tag="pr")
                for i in range(2):
                    nc.tensor.matmul(ps[:, 0:128],
                                     lhsT=wsb[:, i, j * 128:(j + 1) * 128],
                                     rhs=hT[:, i, hslot * 128:hslot * 128 + 128],
                                     start=(i == 0), stop=(i == 1))
                nc.vector.tensor_copy(out=kdst[:, j, kcols:kcols + 128],
                                      in_=ps[:, 0:128])

        def v_proj(wsb, hslot, dst_hx):
            ps = ps_aoT.tile([128, 512], f32, tag="aoT", bufs=1)
            for i in range(2):
                nc.tensor.matmul(ps[:, 0:D],
                                 lhsT=hT[:, i, hslot * 128:hslot * 128 + 128],
                                 rhs=wsb[:, i, :],
                                 start=(i == 0), stop=(i == 1))
            nc.scalar.copy(out=dst_hx,
                           in_=ps[:, 0:D].rearrange("p (h x) -> p h x", h=H))

        def q_proj(wsb, hslot, qdst, qcols):
            ps = ps_pr.tile([128, 512], f32, tag="pr")
            for j in range(2):
                for i in range(2):
                    nc.tensor.matmul(ps[:, j * 128:(j + 1) * 128],
                                     lhsT=wsb[:, i, j * 128:(j + 1) * 128],
                                     rhs=hT[:, i, hslot * 128:hslot * 128 + 128],
                                     start=(i == 0), stop=(i == 1))
            nc.vector.tensor_copy(
                out=qdst[:, :, qcols:qcols + 128],
                in_=ps[:, 0:D].rearrange("p (j c) -> p j c", j=2))

        # ---- group pass: 4 query tiles x 12-key-tile window ----
        def attn_pass(kTw, vw, g, kT4_l, v4_l, qT_l, qoff, expD, woh, xslots):
            """One 4-query-tile attention pass.
            kTw/vw: key window tensors; g: window tile offset; qT_l/qoff:
            query tensor + col offset; expD: DRAM exp table [H,128,4096];
            xslots: x_q slot per local query tile (residual target)."""
            phs = []
            for hh in range(H):
                pb, dt_ = (hh % 2) * 64, hh // 2
                ph = pe_pool.tile([128, EXPW_COLS], bf16, tag="ph")
                phs.append(ph)
                nc.gpsimd.dma_start(out=ph[:, :], in_=expD[hh, :, :])
                for g0, g1, goff, gcols in GROUPS:
                    st = ps_s.tile([128, 1024], f32, tag="st")
                    for ri in range(g0, g1):
                        w, ts, _a, _b = RUNS[ri]
                        ncol = len(ts) * 128
                        q0 = qoff + ts[0] * 128
                        lo = RUNCOLS[ri] - goff
                        if w == "loc4":
                            kl = kT4_l[pb:pb + 64, dt_, :]
                        else:
                            kl = kTw[pb:pb + 64, dt_,
                                     (g + w) * 128:(g + w + 1) * 128]
                        cuts = [0, ncol]
                        if lo < 512 < lo + ncol:
                            cuts = [0, 512 - lo, ncol]
                        for a, b in zip(cuts[:-1], cuts[1:]):
                            nc.tensor.matmul(
                                st[:, lo + a:lo + b], lhsT=kl,
                                rhs=qT_l[pb:pb + 64, dt_, q0 + a:q0 + b],
                                start=True, stop=True)
                    pe = pm_pool.tile([128, 1024], bf16, tag="pe", bufs=3)
                    nc.scalar.activation(out=pe[:, 0:gcols], in_=st[:, 0:gcols],
                                         func=AF.Exp, scale=float(SCALE))
                    nc.vector.tensor_mul(ph[:, goff:goff + gcols],
                                         pe[:, 0:gcols],
                                         ph[:, goff:goff + gcols])
            for hh in range(H):
                ph = phs[hh]
                aoTp = ps_aoT.tile([128, 512], f32, tag="aoT", bufs=1)
                for ri, (w, ts, av_s, av_e) in enumerate(RUNS):
                    ncol = len(ts) * 128
                    q0 = ts[0] * 128
                    if w == "loc4":
                        vv = v4_l[:, hh, :]
                    else:
                        vv = vw[:, g + w, hh, :]
                    nc.tensor.matmul(
                        aoTp[0:65, q0:q0 + ncol], lhsT=vv,
                        rhs=ph[:, RUNCOLS[ri]:RUNCOLS[ri] + ncol],
                        start=av_s, stop=av_e)
                nc.scalar.copy(out=aoT_sb[0:65, hh, :], in_=aoTp[0:65, :])
                # fold 1/den into aoT along the query (free) axis: recip the
                # den row down to partition 0 (partition_broadcast reads
                # physical partition 0), broadcast across dh partitions, mult
                dr = wk2_pool.tile([1, 512], bf16, tag="dr", bufs=1)
                with nc.allow_low_precision(reason="bf16 den recip, 2e-2 gate"):
                    nc.vector.reciprocal(out=dr[0:1, :],
                                         in_=aoT_sb[64:65, hh, :])
                dbc = wk2_pool.tile([64, 512], bf16, tag="dbc")
                nc.gpsimd.partition_broadcast(out_ap=dbc[:], in_ap=dr[0:1, :],
                                              channels=64)
                nc.vector.tensor_mul(aoT_sb[0:64, hh, :],
                                     aoT_sb[0:64, hh, :], dbc[:])
            for t in range(4):
                xo = ps_pr.tile([128, 512], f32, tag="pr")
                for hh in range(H):
                    nc.tensor.matmul(xo[:, 0:D],
                                     lhsT=aoT_sb[0:64, hh, t * 128:(t + 1) * 128],
                                     rhs=woh[:, hh, :], start=(hh == 0),
                                     stop=(hh == H - 1))
                nc.vector.tensor_add(x_q[:, xslots[t], :],
                                     x_q[:, xslots[t], :], xo[:, 0:D])

        # ---- dense pass: one query tile, explicit key-tile list ----
        def dense_pass(kts, kTx_l, vx_l, kT4_l, v4_l, qT_l, qcol, expD,
                       wo_sb, xslot):
            """kts: list of ("c", idx) / ("loc4",); expD [H, 128, nkt*128]."""
            nkt = len(kts)
            nc_cols = nkt * 128
            ao_ps = ps_pr.tile([128, 512], f32, tag="pr")
            for hh in range(H):
                pb, dt_ = (hh % 2) * 64, hh // 2
                ph = pm_pool.tile([128, 1024], bf16, tag="pe", bufs=3)
                nc.gpsimd.dma_start(out=ph[:, 0:nc_cols],
                                    in_=expD[hh, :, 0:nc_cols])
                st = ps_s.tile([128, 1024], f32, tag="st")
                for ki, kt in enumerate(kts):
                    if kt[0] == "loc4":
                        lhsT = kT4_l[pb:pb + 64, dt_, :]
                    else:
                        w = kt[1]
                        lhsT = kTx_l[pb:pb + 64, dt_, w * 128:(w + 1) * 128]
                    nc.tensor.matmul(st[:, ki * 128:(ki + 1) * 128],
                                     lhsT=lhsT,
                                     rhs=qT_l[pb:pb + 64, dt_, qcol:qcol + 128],
                                     start=True, stop=True)
                pe = pm_pool.tile([128, 1024], bf16, tag="pe", bufs=3)
                nc.scalar.activation(out=pe[:, 0:nc_cols], in_=st[:, 0:nc_cols],
                                     func=AF.Exp, scale=float(SCALE))
                nc.vector.tensor_mul(ph[:, 0:nc_cols], pe[:, 0:nc_cols],
                                     ph[:, 0:nc_cols])
                for ki, kt in enumerate(kts):
                    if kt[0] == "loc4":
                        rhs = v4_l[:, hh, :]
                    else:
                        rhs = vx_l[:, kt[1], hh, :]
                    nc.tensor.matmul(ao_ps[:, hh * 65:hh * 65 + 65],
                                     lhsT=ph[:, ki * 128:(ki + 1) * 128],
                                     rhs=rhs, start=(ki == 0),
                                     stop=(ki == nkt - 1))
            rec = wk2_pool.tile([128, 4], f32, tag="rec")
            nc.vector.reciprocal(out=rec[:], in_=ao_ps[:, DH:260:DH + 1])
            ao_sb = wk2_pool.tile([128, D], bf16, tag="aosb")
            for hh in range(H):
                nc.vector.tensor_scalar(
                    out=ao_sb[:, hh * DH:(hh + 1) * DH],
                    in0=ao_ps[:, hh * 65:hh * 65 + DH],
                    scalar1=rec[:, hh:hh + 1], scalar2=None,
                    op0=mybir.AluOpType.mult)
            aoT = wk2_pool.tile([128, 2, 128], bf16, tag="aoT")
            pe_transpose([ao_sb[:]], aoT, 0)
            xo = ps_pr.tile([128, 512], f32, tag="pr")
            for i in range(2):
                nc.tensor.matmul(xo[:, 0:D], lhsT=aoT[:, i, :],
                                 rhs=wo_sb[:, i, :],
                                 start=(i == 0), stop=(i == 1))
            nc.vector.tensor_add(x_q[:, xslot, :], x_q[:, xslot, :], xo[:, 0:D])

        # ---- FFN over x_q slots: all LN+transposes first (one sqrt/gelu
        # table transition), then 5-tile gelu/w2 waves ----
        def ffn(slots, w1sb, w2sb):
            hs = []
            for group in range(0, len(slots), 8):
                gs = slots[group:group + 8]
                ghs = []
                for s in gs:
                    h2 = tmp_pool.tile([128, D], bf16, tag="h2", bufs=4)
                    ghs.append(h2)
                ln_batch([(x_q[:, s, :], ghs[i][:]) for i, s in enumerate(gs)])
                for i in range(0, len(gs), 2):
                    pe_transpose([h[:] for h in ghs[i:i + 2]], hT2,
                                 group + i)
                hs += ghs
            for w0 in range(0, len(slots), 5):
                wslots = slots[w0:w0 + 5]
                n = len(wslots)
                for fj in range(8):
                    c0 = 0
                    while c0 < n * 128:
                        c1 = min(c0 + 512, n * 128)
                        ps = ps_pr.tile([128, 512], f32, tag="pr")
                        for i in range(2):
                            nc.tensor.matmul(
                                ps[:, 0:c1 - c0],
                                lhsT=w1sb[:, i, fj * 128:(fj + 1) * 128],
                                rhs=hT2[:, i, w0 * 128 + c0:w0 * 128 + c1],
                                start=(i == 0), stop=(i == 1))
                        nc.scalar.activation(out=yT[:, fj, c0:c1],
                                             in_=ps[:, 0:c1 - c0],
                                             func=AF.Gelu, scale=1.0)
                        c0 = c1
                for li, s in enumerate(wslots):
                    ps2 = ps_s.tile([128, 1024], f32, tag="st")
                    for fj in range(8):
                        nc.tensor.matmul(ps2[:, 0:D],
                                         lhsT=yT[:, fj, li * 128:(li + 1) * 128],
                                         rhs=w2sb[:, fj, :],
                                         start=(fj == 0), stop=(fj == 7))
                    nc.vector.tensor_add(x_q[:, s, :], x_q[:, s, :],
                                         ps2[:, 0:D])

        # ================= layer 0 =================
        wq0, wk0, wv0 = W[("wq", 0)], W[("wk", 0)], W[("wv", 0)]

        # tile specs: (imgT tensor | None=BOS/EOS, col, hslot, xslot, kdst,
        # kcol, vdst, qcol).  hslots are consecutive 0..30 so PE transposes
        # pack in pairs; processed in waves of 8 with stage-batched LN.
        specs = []
        for i in range(NBK):
            specs.append((imgT, i, i, i - 8 if i >= 8 else None,
                          kT_band, i * 128, v_band[:, i, :, 0:DH],
                          (i - 8) * 128 if i >= 8 else None))
        for j in range(NCK):
            xslot = 12 if j == 7 else (13 if j == 9 else None)
            specs.append((imgTc, j, 20 + j, xslot, kT_cor, j * 128,
                          v_cor[:, j, :, 0:DH],
                          xslot * 128 if xslot is not None else None))
        specs.append((None, 0, 30, 14, kT4, 0, v4[:, :, 0:DH], 14 * 128))

        for w0 in range(0, len(specs), 8):
            wave = specs[w0:w0 + 8]
            xs = []
            for (srcT, col, hslot, xslot, kdst, kcol, vdst, qcol) in wave:
                if srcT is None:
                    nc.vector.tensor_copy(out=x_q[:, 14, :], in_=emb4[:])
                    xs.append(x_q[:, 14, :])
                    continue
                ps = ps_pr.tile([128, 512], f32, tag="pr")
                nc.tensor.matmul(ps[:, 0:D],
                                 lhsT=srcT[:, col * 128:(col + 1) * 128],
                                 rhs=pw[:], start=True, stop=True)
                if xslot is not None:
                    nc.vector.tensor_copy(out=x_q[:, xslot, :], in_=ps[:, 0:D])
                    xs.append(x_q[:, xslot, :])
                else:
                    xt = tmp_pool.tile([128, D], f32, tag="xt")
                    nc.vector.tensor_copy(out=xt[:], in_=ps[:, 0:D])
                    xs.append(xt[:])
            hs = []
            for _ in wave:
                hs.append(tmp_pool.tile([128, D], bf16, tag="h", name="hwv", bufs=4))
            ln_batch([(xs[i], hs[i][:]) for i in range(len(wave))])
            for i in range(0, len(wave), 2):
                pe_transpose([h[:] for h in hs[i:i + 2]], hT, w0 + i)
            for wi, (srcT, col, hslot, xslot, kdst, kcol, vdst, qcol) \
                    in enumerate(wave):
                k_proj(wk0, hslot, kdst, kcol)
                v_proj(wv0, hslot, vdst)
                if qcol is not None:
                    q_proj(wq0, hslot, qT0, qcol)

        # band group passes
        woh0, wo0 = W[("woh", 0)], W[("wo", 0)]
        attn_pass(kT_band, v_band, 0, kT4, v4, qT0, 0, expA_d, woh0,
                  (0, 1, 2, 3))
        load_rest()   # FFN + layer-1 weights stream in behind pass A
        attn_pass(kT_band, v_band, 4, kT4, v4, qT0, 4 * 128, expB_d, woh0,
                  (4, 5, 6, 7))
        attn_pass(kT_band, v_band, 8, kT4, v4, qT0, 8 * 128, expC_d, woh0,
                  (8, 9, 10, 11))
        # corner + BOS/EOS dense passes
        dense_pass([("c", w) for w in KTS29] + [("loc4",)],
                   kT_cor, v_cor, kT4, v4, qT0, 12 * 128, expCr_d[0],
                   wo0, 12)
        dense_pass([("c", w) for w in KTS31] + [("loc4",)],
                   kT_cor, v_cor, kT4, v4, qT0, 13 * 128, expCr_d[1],
                   wo0, 13)
        dense_pass([("loc4",), ("c", 7), ("c", 9)],
                   kT_cor, v_cor, kT4, v4, qT0, 14 * 128, expB4_d,
                   wo0, 14)

        # FFN over all 15 query tiles
        w10, w20 = W[("w1", 0)], W[("w2", 0)]
        ffn(list(range(NQT)), w10, w20)

        # ================= layer 1 =================
        wq1, wk1, wv1 = W[("wq", 1)], W[("wk", 1)], W[("wv", 1)]
        for group in (list(range(8)), list(range(8, NQT))):
            ghs = []
            for s in group:
                ghs.append(tmp_pool.tile([128, D], bf16, tag="h", name="hwv", bufs=4))
            ln_batch([(x_q[:, s, :], ghs[i][:]) for i, s in enumerate(group)])
            for i in range(0, len(group), 2):
                pe_transpose([h[:] for h in ghs[i:i + 2]], hT, group[i])
        for s in range(NBQ):
            k_proj(wk1, s, kT1b, s * 128)
            v_proj(wv1, s, v1b[:, s, :, 0:DH])
        for xi, s in enumerate((12, 13)):
            k_proj(wk1, s, kT1x, xi * 128)
            v_proj(wv1, s, v1x[:, xi, :, 0:DH])
        k_proj(wk1, 14, kT41, 0)
        v_proj(wv1, 14, v41[:, :, 0:DH])
        for t in range(4):
            q_proj(wq1, 8 + t, qT1, t * 128)
        q_proj(wq1, 14, qT1, 4 * 128)

        woh1, wo1 = W[("woh", 1)], W[("wo", 1)]
        attn_pass(kT1b, v1b, 0, kT41, v41, qT1, 0, expC_d, woh1,
                  (8, 9, 10, 11))
        dense_pass([("loc4",), ("c", 0), ("c", 1)],
                   kT1x, v1x, kT41, v41, qT1, 4 * 128, expB4_d,
                   wo1, 14)

        w11, w21 = W[("w1", 1)], W[("w2", 1)]
        ffn([8, 9, 10, 11, 14], w11, w21)

        # ---------------- final LN + output ----------------
        ln_batch([(x_q[:, s, :], o_sb[:, lt, :])
                  for lt, s in enumerate((8, 9, 10, 11, 14))])
        for lt in range(5):
            sync.dma_start(out=out_d[lt * 128:(lt + 1) * 128, :],
                           in_=o_sb[:, lt, :])

    nc.finalize()
    return nc


# ======================= host side =======================

def _patchify(img):
    x = img.reshape(1, 1, GT, PATCH, GT, PATCH, GT, PATCH)
    x = np.einsum("nctphqwr->nthwpqrc", x).reshape(N, PVOL)
    return np.ascontiguousarray(x).astype(np.float32)


def _expA_to_runs(expA):
    """regroup per-(t,ki) blocks [4,H,128,8*128] into RUNS layout [H,128,4096]"""
    expW = np.zeros((H, 128, EXPW_COLS), np.float32)
    for ri, (w, ts, _a, _b) in enumerate(RUNS):
        co = RUNCOLS[ri]
        for t in ts:
            ki = 7 if w == "loc4" else [t, t + 3, t + 4, t + 5, t + 6,
                                        t + 7, t + 8].index(w)
            expW[:, :, co + (t - ts[0]) * 128:co + (t - ts[0] + 1) * 128] = \
                expA[t, :, :, ki * 128:(ki + 1) * 128]
    return expW


def _host_prep(inputs):
    idx = np.asarray(inputs["idx"])
    valid = np.asarray(inputs["valid"])
    geo = np.asarray(inputs["geo_dist"]).astype(np.float32)
    decay = np.asarray(inputs["decay"]).astype(np.float32)
    K = idx.shape[1]
    fv = valid & (idx <= np.arange(L)[:, None])
    bias_lk = geo[None] * decay[:, None, None]          # [H, L, K]

    patches = _patchify(np.asarray(inputs["input_image"]))
    ids = np.asarray(inputs["input_ids"]).reshape(-1)
    et = np.asarray(inputs["embed_tokens"])
    bos_e, eos_e = et[ids[0]], et[ids[-1]]

    emb4 = np.zeros((128, D), np.float32)
    emb4[0] = bos_e
    emb4[1] = eos_e

    # group-pass exp tables: ct(vc) = 4-query-tile table for queries =
    # global patch tiles 4vc..4vc+3 over window tiles 4vc-8..4vc+3
    def build_ct(vc):
        expA = np.zeros((4, H, 128, 8 * 128), np.float32)
        if vc < 0:
            # fully-padded queries attend BOS with weight 1 (finite den)
            expA[:, :, 0, 7 * 128:8 * 128] = 1.0
            return _expA_to_runs(expA)
        base = vc * 512 - 1024
        for lq in range(512):
            gq = 1 + vc * 512 + lq
            t, lcol = lq // 128, lq % 128
            m = fv[gq]
            kr = idx[gq][m].astype(np.int64)
            ev = np.exp(bias_lk[:, gq, m])               # [H, nk]
            bos = kr == 0
            if bos.any():
                expA[t, :, 0, 7 * 128 + lcol] = ev[:, bos][:, 0]
            nb = ~bos
            krn = kr[nb] - 1 - base
            assert np.all((krn >= 0) & (krn < 1536)), (vc, gq)
            w, j = krn // 128, krn % 128
            off = w - t
            ki = np.where(off == 0, 0, off - 2)
            assert np.all(((off == 0) | ((off >= 3) & (off <= 8)))), (vc, gq)
            expA[t, :, j, ki * 128 + lcol] = ev[:, nb].T
        return _expA_to_runs(expA)

    ct = {-2: build_ct(-2).astype(BF16)}   # pad table (all queries padded)
    ct[-1] = ct[-2]
    for vc in range(NCORES):
        ct[vc] = build_ct(vc).astype(BF16)

    # corner-query tables (tiles 29, 31) -- shared across cores
    g2a = {g: a for a, g in enumerate(CKT)}
    expCr = np.zeros((2, H, 128, 8 * 128), np.float32)
    for ti, T in enumerate((29, 31)):
        kts_g = [T - 8, T - 5, T - 4, T - 3, T - 2, T - 1, T]
        for lcol in range(128):
            gq = 1 + T * 128 + lcol
            m = fv[gq]
            kr = idx[gq][m].astype(np.int64)
            ev = np.exp(bias_lk[:, gq, m])
            bos = kr == 0
            if bos.any():
                expCr[ti, :, 0, 7 * 128 + lcol] = ev[:, bos][:, 0]
            nb = ~bos
            krn = kr[nb] - 1
            KT, j = krn // 128, krn % 128
            ki = np.array([kts_g.index(int(x)) for x in KT])
            expCr[ti, :, j, ki * 128 + lcol] = ev[:, nb].T
    expCr = expCr.astype(BF16)

    # BOS/EOS/pad-query table (kts = [loc4, corner7(=29), corner9(=31)])
    expB4 = np.zeros((H, 128, 3 * 128), np.float32)
    expB4[:, 0, 2:128] = 1.0                             # pad queries attend BOS
    for li, gq in ((0, 0), (1, L - 1)):
        for k in range(K):
            if not fv[gq, k]:
                continue
            kr = int(idx[gq, k])
            ev = np.exp(bias_lk[:, gq, k])
            if kr == 0:
                expB4[:, 0, li] = ev
            elif kr == L - 1:
                expB4[:, 1, li] = ev
            else:
                p = kr - 1
                if 3712 <= p < 3840:
                    expB4[:, p - 3712, 1 * 128 + li] = ev
                elif 3968 <= p < 4096:
                    expB4[:, p - 3968, 2 * 128 + li] = ev
                else:
                    raise AssertionError((gq, kr))
    expB4 = expB4.astype(BF16)

    # corner-key image tiles (shared)
    imgTc = np.concatenate([patches[T * 128:(T + 1) * 128].T for T in CKT],
                           axis=1).astype(BF16)          # [64, 1280]

    per_core = []
    for c in range(NCORES):
        imgT = np.zeros((PVOL, NBK * 128), np.float32)
        lo = c * LC - 2048
        s0, s1 = max(0, -lo), min(NBK * 128, N - lo)
        imgT[:, s0:s1] = patches[lo + s0:lo + s1].T
        per_core.append({"imgT": imgT.astype(BF16),
                         "expA": np.ascontiguousarray(ct[c - 2]),
                         "expB": np.ascontiguousarray(ct[c - 1]),
                         "expC": np.ascontiguousarray(ct[c])})

    shared = {
        "imgTc": imgTc,
        "emb4": emb4,
        "expCr": expCr,
        "expB4": expB4,
        "ident": np.eye(128, dtype=np.float32).astype(BF16),
        "patch_w": np.asarray(inputs["patch_w"]).astype(BF16),
    }
    for nm in ("wq", "wk", "wv", "wo", "w1", "w2"):
        shared[nm] = np.asarray(inputs[nm]).astype(BF16)

    # this model instance has trivial LN affine and zero residual biases
    for nm, s_, b_ in (("ln1", inputs["ln1_s"], inputs["ln1_b"]),
                       ("ln2", inputs["ln2_s"], inputs["ln2_b"]),
                       ("lnf", inputs["norm_s"], inputs["norm_b"])):
        assert np.all(np.asarray(s_) == 1.0) and np.all(np.asarray(b_) == 0.0), \
            f"{nm} affine unsupported"
    for nm in ("bo", "b1", "b2", "patch_b"):
        assert np.all(np.asarray(inputs[nm]) == 0.0), f"{nm} nonzero unsupported"

    return per_core, shared


def kernel(**inputs):
    from concourse.bass_utils import run_bass_kernel_spmd

    per_core, shared = _host_prep(inputs)
    if "prog" not in _prog_cache:
        _prog_cache["prog"] = _build_program()
    nc = _prog_cache["prog"]

    in_maps = []
    for c in range(NCORES):
        m = dict(shared)
        m.update(per_core[c])
        in_maps.append(m)
    import os
    trace = bool(os.environ.get("KERNEL_TRACE"))
    res = run_bass_kernel_spmd(nc, in_maps, core_ids=list(range(NCORES)),
                               trace=trace)
    global _last_exec_ns
    _last_exec_ns = res.exec_time_ns

    out = np.zeros((L, D), np.float32)
    for c in range(NCORES):
        out[1 + c * LC:1 + (c + 1) * LC] = res.results[c]["out"][0:LC]
    out[0] = res.results[0]["out"][LC]
    out[L - 1] = res.results[0]["out"][LC + 1]
    return out.reshape(1, L, D)


# revision 3
# speedup vs baseline: 1.0138x; 1.0138x over previous
"""Trainium2 Bass kernel for sparse-attention 3D-ViT (nn_BaseModel_44341242364529).

No-collective design: the layer-1 AllGather of band k/v is replaced by
redundant layer-0 compute.  Each core computes layer 0 (attention + FFN)
for its full 12-tile query band (rows c*512-1024 .. c*512+512), the two
corner tiles EOS attends (29, 31), and the BOS/EOS tile -- so layer-1
k/v for every key any own-query needs is produced locally and no core
ever waits on another (cross-core barrier waits dominated the measured
exec time of the collective version).

Mechanics (inherited from the collective baseline):
- all matmuls bf16, fp32 PSUM accumulation
- multiplicative attention bias: P = exp(scale*S) * expW, expW tables
  host-precomputed (masked slots = 0; fully-padded query rows attend BOS
  with weight 1 so their denominator stays finite); tables are STREAMED
  from DRAM into the P tile and multiplied in place (16MB total, too big
  to keep resident)
- attention runs in 4-query-tile "group passes" over a 12-key-tile
  window (the RUNS table): layer 0 = 3 band passes, layer 1 = 1 pass;
  corner/BOS-EOS queries use small dense per-tile passes
- w-grouped S matmuls (stationary kT tile), transposed AV (stationary
  65-col v tile with ones column for the denominator), per-head wo with
  the softmax denominator folded into the residual update
- LN per-tile (bn_stats -> sqrt+recip -> gpsimd apply -> DMA transposes)
"""

import numpy as np
import ml_dtypes

# model dims (hardcoded per spec)
IMG, PATCH, D, H, NLAYERS, DFF = 64, 4, 256, 4, 2, 1024
GT = IMG // PATCH          # 16
N = GT * GT * GT           # 4096
L = N + 2                  # 4098
DH = D // H                # 64
PVOL = PATCH ** 3          # 64
NCORES = 8
LC = 512                   # real patch rows per core
LLOC = 640                 # padded local rows (5 tiles of 128)
SCALE = 1.0 / np.sqrt(DH)  # 0.125
BF16 = ml_dtypes.bfloat16

NBK = 20                   # band key tiles (global tiles 4c-16 .. 4c+3)
NBQ = 12                   # band query tiles (= band key tiles 8..19)
CKT = [21, 23, 24, 25, 26, 27, 28, 29, 30, 31]   # corner key tiles (global)
NCK = len(CKT)
# corner query tiles 29, 31: key lists in corner-array indices (+ BOS slot)
KTS29 = [0, 2, 3, 4, 5, 6, 7]
KTS31 = [1, 4, 5, 6, 7, 8, 9]
NQT = 15                   # x_q slots: 12 band + 2 corner + 1 BOS/EOS

# w-grouped attention runs for a 4-query-tile group over a 12-tile key
# window: (w, tlist, av_start, av_stop).  w numeric = window tile, "loc4"
# = the local BOS/EOS tile (BOS key at partition 0).
RUNS = [("loc4", (0, 1, 2, 3), True, False),
        (8, (0, 1, 2, 3), False, False), (9, (1, 2, 3), False, False),
        (10, (2, 3), False, False), (11, (3,), False, False),
        (0, (0,), False, False), (1, (1,), False, False), (2, (2,), False, False),
        (3, (0,), False, False), (3, (3,), False, False),
        (4, (0, 1), False, False), (5, (0, 1, 2), False, False),
        (6, (0, 1, 2, 3), False, False), (7, (0, 1, 2, 3), False, True)]
RUNCOLS = []
_off = 0
for _w, _ts, _a, _b in RUNS:
    RUNCOLS.append(_off)
    _off += len(_ts) * 128
EXPW_COLS = _off  # 4096

# run groups for coarse exp/mult: contiguous run ranges, <=1024 cols each
GROUPS = []
_g0 = 0
for _ri in range(len(RUNS) + 1):
    if (_ri == len(RUNS) or _ri == 5
            or (RUNCOLS[_ri] - RUNCOLS[_g0]) + len(RUNS[_ri][1]) * 128 > 1024):
        GROUPS.append((_g0, _ri, RUNCOLS[_g0],
                       (RUNCOLS[_ri - 1] + len(RUNS[_ri - 1][1]) * 128) - RUNCOLS[_g0]))
        _g0 = _ri
        if _ri == len(RUNS):
            break


_prog_cache = {}


def _build_program():
    import concourse.bass as bass
    import concourse.bacc as bacc
    import concourse.tile as tile
    from concourse import mybir

    f32 = mybir.dt.float32
    bf16 = mybir.dt.bfloat16
    AF = mybir.ActivationFunctionType
    nc = bacc.Bacc("TRN2", target_bir_lowering=False, debug=False,
                   num_devices=NCORES)

    def din(name, shape, dt_=bf16):
        return nc.declare_dram_parameter(name, list(shape), dt_, isOutput=False)

    imgT_d = din("imgT", [PVOL, NBK * 128])
    imgTc_d = din("imgTc", [PVOL, NCK * 128])
    emb4_d = din("emb4", [128, D], f32)
    pw_d = din("patch_w", [PVOL, D])
    wq_d = din("wq", [NLAYERS, D, D])
    wk_d = din("wk", [NLAYERS, D, D])
    wv_d = din("wv", [NLAYERS, D, D])
    wo_d = din("wo", [NLAYERS, D, D])
    w1_d = din("w1", [NLAYERS, D, DFF])
    w2_d = din("w2", [NLAYERS, DFF, D])
    expA_d = din("expA", [H, 128, EXPW_COLS])   # band pass A (queries 0..3)
    expB_d = din("expB", [H, 128, EXPW_COLS])   # band pass B (queries 4..7)
    expC_d = din("expC", [H, 128, EXPW_COLS])   # band pass C / layer-1 pass
    expCr_d = din("expCr", [2, H, 128, 8 * 128])  # corner queries 29, 31
    expB4_d = din("expB4", [H, 128, 3 * 128])     # BOS/EOS/pad queries
    ident_d = din("ident", [128, 128])            # PE-transpose identity
    out_d = nc.declare_dram_parameter("out", [LLOC, D], f32, isOutput=True)

    from contextlib import ExitStack
    with tile.TileContext(nc) as tc, ExitStack() as ctx:
        sing = ctx.enter_context(tc.tile_pool(name="sing", bufs=1))
        wk_pool = ctx.enter_context(tc.tile_pool(name="wrk", bufs=1))
        wk2_pool = ctx.enter_context(tc.tile_pool(name="wrk2", bufs=2))
        tmp_pool = ctx.enter_context(tc.tile_pool(name="tmp", bufs=8))
        pe_pool = ctx.enter_context(tc.tile_pool(name="pexp", bufs=4))
        pm_pool = ctx.enter_context(tc.tile_pool(name="pmul", bufs=4))
        ps_s = ctx.enter_context(tc.tile_pool(name="pss", bufs=2, space="PSUM"))
        ps_aoT = ctx.enter_context(tc.tile_pool(name="pst", bufs=2, space="PSUM"))
        ps_pr = ctx.enter_context(tc.tile_pool(name="psp", bufs=2, space="PSUM"))

        sync = nc.sync

        # ---------------- constants / weights ----------------
        # DMA queue drains serially: request in first-use order
        pw = sing.tile([PVOL, D], bf16, tag="pw")
        sync.dma_start(out=pw[:], in_=pw_d[:, :])
        ident = sing.tile([128, 128], bf16, tag="ident")
        sync.dma_start(out=ident[:], in_=ident_d[:, :])
        imgT = sing.tile([PVOL, NBK * 128], bf16, tag="imgT")
        sync.dma_start(out=imgT[:], in_=imgT_d[:, :])
        imgTc = sing.tile([PVOL, NCK * 128], bf16, tag="imgTc")
        sync.dma_start(out=imgTc[:], in_=imgTc_d[:, :])
        emb4 = sing.tile([128, D], f32, tag="emb4")
        sync.dma_start(out=emb4[:], in_=emb4_d[:, :])

        W = {}

        def loadw(nm, dt_, l):
            kd = 8 if nm == "w2" else 2
            t_ = sing.tile([128, kd, dt_.shape[2]], bf16, tag=f"{nm}{l}")
            sync.dma_start(out=t_[:], in_=dt_[l].rearrange("(k p) n -> p k n", p=128))
            W[(nm, l)] = t_

        def loadwoh(l):
            t_ = sing.tile([64, H, D], bf16, tag=f"woh{l}")
            sync.dma_start(out=t_[:], in_=wo_d[l].rearrange("(h p) n -> p h n", p=64))
            W[("woh", l)] = t_

        for nm, dt_ in (("wk", wk_d), ("wv", wv_d), ("wq", wq_d)):
            loadw(nm, dt_, 0)
        loadwoh(0)
        loadw("wo", wo_d, 0)

        def load_rest():
            for nm, dt_ in (("w1", w1_d), ("w2", w2_d)):
                loadw(nm, dt_, 0)
            for nm, dt_ in (("wk", wk_d), ("wv", wv_d), ("wq", wq_d),
                            ("wo", wo_d), ("w1", w1_d), ("w2", w2_d)):
                loadw(nm, dt_, 1)
            loadwoh(1)

        eps_sb = sing.tile([128, 1], f32, tag="eps")
        nc.vector.memset(eps_sb[:], 1e-5)

        # ---------------- persistent activations ----------------
        x_q = wk_pool.tile([128, NQT, D], f32, tag="xq")
        hT = wk_pool.tile([128, 2, 31 * 128], bf16, tag="hT")
        qT0 = wk_pool.tile([128, 2, NQT * 128], bf16, tag="qT0")
        qT1 = wk_pool.tile([128, 2, 5 * 128], bf16, tag="qT1")
        kT_band = wk_pool.tile([128, 2, NBK * 128], bf16, tag="kband")
        v_band = wk_pool.tile([128, NBK, H, DH + 1], bf16, tag="vband")
        kT_cor = wk_pool.tile([128, 2, NCK * 128], bf16, tag="kcor")
        v_cor = wk_pool.tile([128, NCK, H, DH + 1], bf16, tag="vcor")
        kT4 = wk_pool.tile([128, 2, 128], bf16, tag="kT4")
        v4 = wk_pool.tile([128, H, DH + 1], bf16, tag="v4")
        kT1b = wk_pool.tile([128, 2, NBQ * 128], bf16, tag="k1b")
        v1b = wk_pool.tile([128, NBQ, H, DH + 1], bf16, tag="v1b")
        kT1x = wk_pool.tile([128, 2, 256], bf16, tag="k1x")
        v1x = wk_pool.tile([128, 2, H, DH + 1], bf16, tag="v1x")
        kT41 = wk_pool.tile([128, 2, 128], bf16, tag="kT41")
        v41 = wk_pool.tile([128, H, DH + 1], bf16, tag="v41")
        hT2 = wk_pool.tile([128, 2, NQT * 128], bf16, tag="hT2")
        yT = wk_pool.tile([128, 8, 5 * 128], bf16, tag="yT")
        stats = wk_pool.tile([128, 32, 6], f32, tag="stats")
        mv = wk_pool.tile([128, 32, 2], f32, tag="mv")
        rstd = wk_pool.tile([128, 32], f32, tag="rstd")
        o_sb = wk_pool.tile([128, 5, D], f32, tag="osb")
        aoT_sb = wk_pool.tile([128, H, 512], bf16, tag="aoTsb")

        # ones columns for the denominator trick
        nc.vector.memset(v_band[:, :, :, DH:DH + 1], 1.0)
        nc.vector.memset(v_cor[:, :, :, DH:DH + 1], 1.0)
        nc.vector.memset(v4[:, :, DH:DH + 1], 1.0)
        nc.vector.memset(v1b[:, :, :, DH:DH + 1], 1.0)
        nc.vector.memset(v1x[:, :, :, DH:DH + 1], 1.0)
        nc.vector.memset(v41[:, :, DH:DH + 1], 1.0)

        # ---------------- helpers ----------------
        _ln_i = [0]

        def ln_batch(pairs):
            """pairs: [(src f32 [128,D], dst bf16/f32 [128,D])], stage-batched
            (all stats, one sqrt, one recip, all applies) so each engine's
            FIFO streams independent work instead of per-tile chains."""
            n = len(pairs)
            assert n <= 16
            i0 = 0 if _ln_i[0] % 32 + n > 32 else _ln_i[0] % 32
            _ln_i[0] = i0 + n
            for i, (src, dst) in enumerate(pairs):
                nc.vector.bn_stats(out=stats[:, i0 + i, :], in_=src)
                nc.vector.bn_aggr(out=mv[:, i0 + i, :], in_=stats[:, i0 + i, :])
            nc.scalar.activation(out=rstd[:, i0:i0 + n],
                                 in_=mv[:, i0:i0 + n, 1],
                                 func=AF.Sqrt, bias=eps_sb[:], scale=1.0)
            nc.vector.reciprocal(out=rstd[:, i0:i0 + n], in_=rstd[:, i0:i0 + n])
            for i, (src, dst) in enumerate(pairs):
                nc.gpsimd.tensor_scalar(out=dst, in0=src,
                                        scalar1=mv[:, i0 + i, 0:1],
                                        scalar2=rstd[:, i0 + i:i0 + i + 1],
                                        op0=mybir.AluOpType.subtract,
                                        op1=mybir.AluOpType.mult)

        def pe_transpose(srcs, dst, slot0):
            """PE-transpose <=2 row-major bf16 [128,D] tiles into feature-major
            hT slots (keeps the DMA queue free; bf16 PSUM pass-through)."""
            n = len(srcs)
            ps = ps_aoT.tile([128, 1024], bf16, tag="tr", bufs=1)
            with nc.allow_low_precision(reason="transpose is pass-through"):
                for i, src in enumerate(srcs):
                    for j in range(2):
                        nc.tensor.matmul(
                            ps[:, (2 * i + j) * 128:(2 * i + j + 1) * 128],
                            lhsT=src[:, j * 128:(j + 1) * 128], rhs=ident[:],
                            is_transpose=True, start=True, stop=True)
            nc.scalar.copy(
                out=dst[:, :, slot0 * 128:(slot0 + n) * 128].rearrange(
                    "p j (t c) -> p j t c", t=n),
                in_=ps[:, 0:n * 256].rearrange("p (t j c) -> p j t c",
                                               j=2, c=128))

        def k_proj(wsb, hslot, kdst, kcols):
            """kdst[:, j, kcols:+128] = wk_j^T @ h  for j in 0..1"""
            for j in range(2):
                ps = ps_pr.tile([128, 512], f32, tag="pr")
                for i in range(2):
                    nc.tensor.matmul(ps[:, 0:128],
                                     lhsT=wsb[:, i, j * 128:(j + 1) * 128],
                                     rhs=hT[:, i, hslot * 128:hslot * 128 + 128],
                                     start=(i == 0), stop=(i == 1))
                nc.vector.tensor_copy(out=kdst[:, j, kcols:kcols + 128],
                                      in_=ps[:, 0:128])

        def v_proj(wsb, hslot, dst_hx):
            ps = ps_aoT.tile([128, 512], f32, tag="aoT", bufs=1)
            for i in range(2):
                nc.tensor.matmul(ps[:, 0:D],
                                 lhsT=hT[:, i, hslot * 128:hslot * 128 + 128],
                                 rhs=wsb[:, i, :],
                                 start=(i == 0), stop=(i == 1))
            nc.scalar.copy(out=dst_hx,
                           in_=ps[:, 0:D].rearrange("p (h x) -> p h x", h=H))

        def q_proj(wsb, hslot, qdst, qcols):
            ps = ps_pr.tile([128, 512], f32, tag="pr")
            for j in range(2):
                for i in range(2):
                    nc.tensor.matmul(ps[:, j * 128:(j + 1) * 128],
                                     lhsT=wsb[:, i, j * 128:(j + 1) * 128],
                                     rhs=hT[:, i, hslot * 128:hslot * 128 + 128],
                                     start=(i == 0), stop=(i == 1))
            nc.vector.tensor_copy(
                out=qdst[:, :, qcols:qcols + 128],
                in_=ps[:, 0:D].rearrange("p (j c) -> p j c", j=2))

        # ---- group pass: 4 query tiles x 12-key-tile window ----
        def attn_pass(kTw, vw, g, kT4_l, v4_l, qT_l, qoff, expD, woh, xslots):
            """One 4-query-tile attention pass.
            kTw/vw: key window tensors; g: window tile offset; qT_l/qoff:
            query tensor + col offset; expD: DRAM exp table [H,128,4096];
            xslots: x_q slot per local query tile (residual target)."""
            phs = []
            for hh in range(H):
                pb, dt_ = (hh % 2) * 64, hh // 2
                ph = pe_pool.tile([128, EXPW_COLS], bf16, tag="ph")
                phs.append(ph)
                nc.gpsimd.dma_start(out=ph[:, :], in_=expD[hh, :, :])
                for g0, g1, goff, gcols in GROUPS:
                    st = ps_s.tile([128, 1024], f32, tag="st")
                    for ri in range(g0, g1):
                        w, ts, _a, _b = RUNS[ri]
                        ncol = len(ts) * 128
                        q0 = qoff + ts[0] * 128
                        lo = RUNCOLS[ri] - goff
                        if w == "loc4":
                            kl = kT4_l[pb:pb + 64, dt_, :]
                        else:
                            kl = kTw[pb:pb + 64, dt_,
                                     (g + w) * 128:(g + w + 1) * 128]
                        cuts = [0, ncol]
                        if lo < 512 < lo + ncol:
                            cuts = [0, 512 - lo, ncol]
                        for a, b in zip(cuts[:-1], cuts[1:]):
                            nc.tensor.matmul(
                                st[:, lo + a:lo + b], lhsT=kl,
                                rhs=qT_l[pb:pb + 64, dt_, q0 + a:q0 + b],
                                start=True, stop=True)
                    pe = pm_pool.tile([128, 1024], bf16, tag="pe", bufs=3)
                    nc.scalar.activation(out=pe[:, 0:gcols], in_=st[:, 0:gcols],
                                         func=AF.Exp, scale=float(SCALE))
                    nc.vector.tensor_mul(ph[:, goff:goff + gcols],
                                         pe[:, 0:gcols],
                                         ph[:, goff:goff + gcols])
            for hh in range(H):
                ph = phs[hh]
                aoTp = ps_aoT.tile([128, 512], f32, tag="aoT", bufs=1)
                for ri, (w, ts, av_s, av_e) in enumerate(RUNS):
                    ncol = len(ts) * 128
                    q0 = ts[0] * 128
                    if w == "loc4":
                        vv = v4_l[:, hh, :]
                    else:
                        vv = vw[:, g + w, hh, :]
                    nc.tensor.matmul(
                        aoTp[0:65, q0:q0 + ncol], lhsT=vv,
                        rhs=ph[:, RUNCOLS[ri]:RUNCOLS[ri] + ncol],
                        start=av_s, stop=av_e)
                nc.scalar.copy(out=aoT_sb[0:65, hh, :], in_=aoTp[0:65, :])
                # fold 1/den into aoT along the query (free) axis: recip the
                # den row down to partition 0 (partition_broadcast reads
                # physical partition 0), broadcast across dh partitions, mult
                dr = wk2_pool.tile([1, 512], bf16, tag="dr", bufs=1)
                with nc.allow_low_precision(reason="bf16 den recip, 2e-2 gate"):
                    nc.vector.reciprocal(out=dr[0:1, :],
                                         in_=aoT_sb[64:65, hh, :])
                dbc = wk2_pool.tile([64, 512], bf16, tag="dbc")
                nc.gpsimd.partition_broadcast(out_ap=dbc[:], in_ap=dr[0:1, :],
                                              channels=64)
                nc.vector.tensor_mul(aoT_sb[0:64, hh, :],
                                     aoT_sb[0:64, hh, :], dbc[:])
            for t in range(4):
                xo = ps_pr.tile([128, 512], f32, tag="pr")
                for hh in range(H):
                    nc.tensor.matmul(xo[:, 0:D],
                                     lhsT=aoT_sb[0:64, hh, t * 128:(t + 1) * 128],
                                     rhs=woh[:, hh, :], start=(hh == 0),
                                     stop=(hh == H - 1))
                nc.vector.tensor_add(x_q[:, xslots[t], :],
                                     x_q[:, xslots[t], :], xo[:, 0:D])

        # ---- dense pass: one query tile, explicit key-tile list ----
        def dense_pass(kts, kTx_l, vx_l, kT4_l, v4_l, qT_l, qcol, expD,
                       wo_sb, xslot):
            """kts: list of ("c", idx) / ("loc4",); expD [H, 128, nkt*128]."""
            nkt = len(kts)
            nc_cols = nkt * 128
            ao_ps = ps_pr.tile([128, 512], f32, tag="pr")
            for hh in range(H):
                pb, dt_ = (hh % 2) * 64, hh // 2
                ph = pm_pool.tile([128, 1024], bf16, tag="pe", bufs=3)
                nc.gpsimd.dma_start(out=ph[:, 0:nc_cols],
                                    in_=expD[hh, :, 0:nc_cols])
                st = ps_s.tile([128, 1024], f32, tag="st")
                for ki, kt in enumerate(kts):
                    if kt[0] == "loc4":
                        lhsT = kT4_l[pb:pb + 64, dt_, :]
                    else:
                        w = kt[1]
                        lhsT = kTx_l[pb:pb + 64, dt_, w * 128:(w + 1) * 128]
                    nc.tensor.matmul(st[:, ki * 128:(ki + 1) * 128],
                                     lhsT=lhsT,
                                     rhs=qT_l[pb:pb + 64, dt_, qcol:qcol + 128],
                                     start=True, stop=True)
                pe = pm_pool.tile([128, 1024], bf16, tag="pe", bufs=3)
                nc.scalar.activation(out=pe[:, 0:nc_cols], in_=st[:, 0:nc_cols],
                                     func=AF.Exp, scale=float(SCALE))
                nc.vector.tensor_mul(ph[:, 0:nc_cols], pe[:, 0:nc_cols],
                                     ph[:, 0:nc_cols])
                for ki, kt in enumerate(kts):
                    if kt[0] == "loc4":
                        rhs = v4_l[:, hh, :]
                    else:
                        rhs = vx_l[:, kt[1], hh, :]
                    nc.tensor.matmul(ao_ps[:, hh * 65:hh * 65 + 65],
                                     lhsT=ph[:, ki * 128:(ki + 1) * 128],
                                     rhs=rhs, start=(ki == 0),
                                     stop=(ki == nkt - 1))
            rec = wk2_pool.tile([128, 4], f32, tag="rec")
            nc.vector.reciprocal(out=rec[:], in_=ao_ps[:, DH:260:DH + 1])
            ao_sb = wk2_pool.tile([128, D], bf16, tag="aosb")
            for hh in range(H):
                nc.vector.tensor_scalar(
                    out=ao_sb[:, hh * DH:(hh + 1) * DH],
                    in0=ao_ps[:, hh * 65:hh * 65 + DH],
                    scalar1=rec[:, hh:hh + 1], scalar2=None,
                    op0=mybir.AluOpType.mult)
            aoT = wk2_pool.tile([128, 2, 128], bf16, tag="aoT")
            pe_transpose([ao_sb[:]], aoT, 0)
            xo = ps_pr.tile([128, 512], f32, tag="pr")
            for i in range(2):
                nc.tensor.matmul(xo[:, 0:D], lhsT=aoT[:, i, :],
                                 rhs=wo_sb[:, i, :],
                                 start=(i == 0), stop=(i == 1))
            nc.vector.tensor_add(x_q[:, xslot, :], x_q[:, xslot, :], xo[:, 0:D])

        # ---- FFN over x_q slots: all LN+transposes first (one sqrt/gelu
        # table transition), then 5-tile gelu/w2 waves ----
        def ffn(slots, w1sb, w2sb):
            hs = []
            for group in range(0, len(slots), 8):
                gs = slots[group:group + 8]
                ghs = []
                for s in gs:
                    h2 = tmp_pool.tile([128, D], bf16, tag="h2", bufs=4)
                    ghs.append(h2)
                ln_batch([(x_q[:, s, :], ghs[i][:]) for i, s in enumerate(gs)])
                for i in range(0, len(gs), 2):
                    pe_transpose([h[:] for h in ghs[i:i + 2]], hT2,
                                 group + i)
                hs += ghs
            for w0 in range(0, len(slots), 5):
                wslots = slots[w0:w0 + 5]
                n = len(wslots)
                for fj in range(8):
                    c0 = 0
                    while c0 < n * 128:
                        c1 = min(c0 + 512, n * 128)
                        ps = ps_pr.tile([128, 512], f32, tag="pr")
                        for i in range(2):
                            nc.tensor.matmul(
                                ps[:, 0:c1 - c0],
                                lhsT=w1sb[:, i, fj * 128:(fj + 1) * 128],
                                rhs=hT2[:, i, w0 * 128 + c0:w0 * 128 + c1],
                                start=(i == 0), stop=(i == 1))
                        nc.scalar.activation(out=yT[:, fj, c0:c1],
                                             in_=ps[:, 0:c1 - c0],
                                             func=AF.Gelu, scale=1.0)
                        c0 = c1
                for li, s in enumerate(wslots):
                    ps2 = ps_s.tile([128, 1024], f32, tag="st")
                    for fj in range(8):
                        nc.tensor.matmul(ps2[:, 0:D],
                                         lhsT=yT[:, fj, li * 128:(li + 1) * 128],
                                         rhs=w2sb[:, fj, :],
                                         start=(fj == 0), stop=(fj == 7))
                    nc.vector.tensor_add(x_q[:, s, :], x_q[:, s, :],
                                         ps2[:, 0:D])

        # ================= layer 0 =================
        wq0, wk0, wv0 = W[("wq", 0)], W[("wk", 0)], W[("wv", 0)]

        # tile specs: (imgT tensor | None=BOS/EOS, col, hslot, xslot, kdst,
        # kcol, vdst, qcol).  hslots are consecutive 0..30 so PE transposes
        # pack in pairs; processed in waves of 8 with stage-batched LN.
        specs = []
        for i in range(NBK):
            specs.append((imgT, i, i, i - 8 if i >= 8 else None,
                          kT_band, i * 128, v_band[:, i, :, 0:DH],
                          (i - 8) * 128 if i >= 8 else None))
        for j in range(NCK):
            xslot = 12 if j == 7 else (13 if j == 9 else None)
            specs.append((imgTc, j, 20 + j, xslot, kT_cor, j * 128,
                          v_cor[:, j, :, 0:DH],
                          xslot * 128 if xslot is not None else None))
        specs.append((None, 0, 30, 14, kT4, 0, v4[:, :, 0:DH], 14 * 128))

        for w0 in range(0, len(specs), 8):
            wave = specs[w0:w0 + 8]
            xs = []
            for (srcT, col, hslot, xslot, kdst, kcol, vdst, qcol) in wave:
                if srcT is None:
                    nc.vector.tensor_copy(out=x_q[:, 14, :], in_=emb4[:])
                    xs.append(x_q[:, 14, :])
                    continue
                ps = ps_pr.tile([128, 512], f32, tag="pr")
                nc.tensor.matmul(ps[:, 0:D],
                                 lhsT=srcT[:, col * 128:(col + 1) * 128],
                                 rhs=pw[:], start=True, stop=True)
                if xslot is not None:
                    nc.scalar.copy(out=x_q[:, xslot, :], in_=ps[:, 0:D])
                    xs.append(x_q[:, xslot, :])
                else:
                    xt = tmp_pool.tile([128, D], f32, tag="xt")
                    nc.scalar.copy(out=xt[:], in_=ps[:, 0:D])
                    xs.append(xt[:])
            hs = []
            for _ in wave:
                hs.append(tmp_pool.tile([128, D], bf16, tag="h", name="hwv", bufs=4))
            ln_batch([(xs[i], hs[i][:]) for i in range(len(wave))])
            for i in range(0, len(wave), 2):
                pe_transpose([h[:] for h in hs[i:i + 2]], hT, w0 + i)
            for wi, (srcT, col, hslot, xslot, kdst, kcol, vdst, qcol) \
                    in enumerate(wave):
                k_proj(wk0, hslot, kdst, kcol)
                v_proj(wv0, hslot, vdst)
                if qcol is not None:
                    q_proj(wq0, hslot, qT0, qcol)

        # band group passes
        woh0, wo0 = W[("woh", 0)], W[("wo", 0)]
        attn_pass(kT_band, v_band, 0, kT4, v4, qT0, 0, expA_d, woh0,
                  (0, 1, 2, 3))
        load_rest()   # FFN + layer-1 weights stream in behind pass A
        attn_pass(kT_band, v_band, 4, kT4, v4, qT0, 4 * 128, expB_d, woh0,
                  (4, 5, 6, 7))
        attn_pass(kT_band, v_band, 8, kT4, v4, qT0, 8 * 128, expC_d, woh0,
                  (8, 9, 10, 11))
        # corner + BOS/EOS dense passes
        dense_pass([("c", w) for w in KTS29] + [("loc4",)],
                   kT_cor, v_cor, kT4, v4, qT0, 12 * 128, expCr_d[0],
                   wo0, 12)
        dense_pass([("c", w) for w in KTS31] + [("loc4",)],
                   kT_cor, v_cor, kT4, v4, qT0, 13 * 128, expCr_d[1],
                   wo0, 13)
        dense_pass([("loc4",), ("c", 7), ("c", 9)],
                   kT_cor, v_cor, kT4, v4, qT0, 14 * 128, expB4_d,
                   wo0, 14)
        w10, w20 = W[("w1", 0)], W[("w2", 0)]
        ffn(list(range(NQT)), w10, w20)

        # ================= layer 1 =================
        wq1, wk1, wv1 = W[("wq", 1)], W[("wk", 1)], W[("wv", 1)]
        for group in (list(range(8)), list(range(8, NQT))):
            ghs = []
            for s in group:
                ghs.append(tmp_pool.tile([128, D], bf16, tag="h", name="hwv", bufs=4))
            ln_batch([(x_q[:, s, :], ghs[i][:]) for i, s in enumerate(group)])
            for i in range(0, len(group), 2):
                pe_transpose([h[:] for h in ghs[i:i + 2]], hT, group[i])
        for s in range(NBQ):
            k_proj(wk1, s, kT1b, s * 128)
            v_proj(wv1, s, v1b[:, s, :, 0:DH])
        for xi, s in enumerate((12, 13)):
            k_proj(wk1, s, kT1x, xi * 128)
            v_proj(wv1, s, v1x[:, xi, :, 0:DH])
        k_proj(wk1, 14, kT41, 0)
        v_proj(wv1, 14, v41[:, :, 0:DH])
        for t in range(4):
            q_proj(wq1, 8 + t, qT1, t * 128)
        q_proj(wq1, 14, qT1, 4 * 128)

        woh1, wo1 = W[("woh", 1)], W[("wo", 1)]
        attn_pass(kT1b, v1b, 0, kT41, v41, qT1, 0, expC_d, woh1,
                  (8, 9, 10, 11))
        dense_pass([("loc4",), ("c", 0), ("c", 1)],
                   kT1x, v1x, kT41, v41, qT1, 4 * 128, expB4_d,
                   wo1, 14)

        w11, w21 = W[("w1", 1)], W[("w2", 1)]
        ffn([8, 9, 10, 11, 14], w11, w21)

        # ---------------- final LN + output ----------------
        ln_batch([(x_q[:, s, :], o_sb[:, lt, :])
                  for lt, s in enumerate((8, 9, 10, 11, 14))])
        for lt in range(5):
            sync.dma_start(out=out_d[lt * 128:(lt + 1) * 128, :],
                           in_=o_sb[:, lt, :])

    nc.finalize()
    return nc


# ======================= host side =======================

def _patchify(img):
    x = img.reshape(1, 1, GT, PATCH, GT, PATCH, GT, PATCH)
    x = np.einsum("nctphqwr->nthwpqrc", x).reshape(N, PVOL)
    return np.ascontiguousarray(x).astype(np.float32)


def _expA_to_runs(expA):
    """regroup per-(t,ki) blocks [4,H,128,8*128] into RUNS layout [H,128,4096]"""
    expW = np.zeros((H, 128, EXPW_COLS), np.float32)
    for ri, (w, ts, _a, _b) in enumerate(RUNS):
        co = RUNCOLS[ri]
        for t in ts:
            ki = 7 if w == "loc4" else [t, t + 3, t + 4, t + 5, t + 6,
                                        t + 7, t + 8].index(w)
            expW[:, :, co + (t - ts[0]) * 128:co + (t - ts[0] + 1) * 128] = \
                expA[t, :, :, ki * 128:(ki + 1) * 128]
    return expW


def _host_prep(inputs):
    idx = np.asarray(inputs["idx"])
    valid = np.asarray(inputs["valid"])
    geo = np.asarray(inputs["geo_dist"]).astype(np.float32)
    decay = np.asarray(inputs["decay"]).astype(np.float32)
    K = idx.shape[1]
    fv = valid & (idx <= np.arange(L)[:, None])
    bias_lk = geo[None] * decay[:, None, None]          # [H, L, K]

    patches = _patchify(np.asarray(inputs["input_image"]))
    ids = np.asarray(inputs["input_ids"]).reshape(-1)
    et = np.asarray(inputs["embed_tokens"])
    bos_e, eos_e = et[ids[0]], et[ids[-1]]

    emb4 = np.zeros((128, D), np.float32)
    emb4[0] = bos_e
    emb4[1] = eos_e

    # group-pass exp tables: ct(vc) = 4-query-tile table for queries =
    # global patch tiles 4vc..4vc+3 over window tiles 4vc-8..4vc+3
    def build_ct(vc):
        expA = np.zeros((4, H, 128, 8 * 128), np.float32)
        if vc < 0:
            # fully-padded queries attend BOS with weight 1 (finite den)
            expA[:, :, 0, 7 * 128:8 * 128] = 1.0
            return _expA_to_runs(expA)
        base = vc * 512 - 1024
        for lq in range(512):
            gq = 1 + vc * 512 + lq
            t, lcol = lq // 128, lq % 128
            m = fv[gq]
            kr = idx[gq][m].astype(np.int64)
            ev = np.exp(bias_lk[:, gq, m])               # [H, nk]
            bos = kr == 0
            if bos.any():
                expA[t, :, 0, 7 * 128 + lcol] = ev[:, bos][:, 0]
            nb = ~bos
            krn = kr[nb] - 1 - base
            assert np.all((krn >= 0) & (krn < 1536)), (vc, gq)
            w, j = krn // 128, krn % 128
            off = w - t
            ki = np.where(off == 0, 0, off - 2)
            assert np.all(((off == 0) | ((off >= 3) & (off <= 8)))), (vc, gq)
            expA[t, :, j, ki * 128 + lcol] = ev[:, nb].T
        return _expA_to_runs(expA)

    ct = {-2: build_ct(-2).astype(BF16)}   # pad table (all queries padded)
    ct[-1] = ct[-2]
    for vc in range(NCORES):
        ct[vc] = build_ct(vc).astype(BF16)

    # corner-query tables (tiles 29, 31) -- shared across cores
    g2a = {g: a for a, g in enumerate(CKT)}
    expCr = np.zeros((2, H, 128, 8 * 128), np.float32)
    for ti, T in enumerate((29, 31)):
        kts_g = [T - 8, T - 5, T - 4, T - 3, T - 2, T - 1, T]
        for lcol in range(128):
            gq = 1 + T * 128 + lcol
            m = fv[gq]
            kr = idx[gq][m].astype(np.int64)
            ev = np.exp(bias_lk[:, gq, m])
            bos = kr == 0
            if bos.any():
                expCr[ti, :, 0, 7 * 128 + lcol] = ev[:, bos][:, 0]
            nb = ~bos
            krn = kr[nb] - 1
            KT, j = krn // 128, krn % 128
            ki = np.array([kts_g.index(int(x)) for x in KT])
            expCr[ti, :, j, ki * 128 + lcol] = ev[:, nb].T
    expCr = expCr.astype(BF16)

    # BOS/EOS/pad-query table (kts = [loc4, corner7(=29), corner9(=31)])
    expB4 = np.zeros((H, 128, 3 * 128), np.float32)
    expB4[:, 0, 2:128] = 1.0                             # pad queries attend BOS
    for li, gq in ((0, 0), (1, L - 1)):
        for k in range(K):
            if not fv[gq, k]:
                continue
            kr = int(idx[gq, k])
            ev = np.exp(bias_lk[:, gq, k])
            if kr == 0:
                expB4[:, 0, li] = ev
            elif kr == L - 1:
                expB4[:, 1, li] = ev
            else:
                p = kr - 1
                if 3712 <= p < 3840:
                    expB4[:, p - 3712, 1 * 128 + li] = ev
                elif 3968 <= p < 4096:
                    expB4[:, p - 3968, 2 * 128 + li] = ev
                else:
                    raise AssertionError((gq, kr))
    expB4 = expB4.astype(BF16)

    # corner-key image tiles (shared)
    imgTc = np.concatenate([patches[T * 128:(T + 1) * 128].T for T in CKT],
                           axis=1).astype(BF16)          # [64, 1280]

    per_core = []
    for c in range(NCORES):
        imgT = np.zeros((PVOL, NBK * 128), np.float32)
        lo = c * LC - 2048
        s0, s1 = max(0, -lo), min(NBK * 128, N - lo)
        imgT[:, s0:s1] = patches[lo + s0:lo + s1].T
        per_core.append({"imgT": imgT.astype(BF16),
                         "expA": np.ascontiguousarray(ct[c - 2]),
                         "expB": np.ascontiguousarray(ct[c - 1]),
                         "expC": np.ascontiguousarray(ct[c])})

    shared = {
        "imgTc": imgTc,
        "emb4": emb4,
        "expCr": expCr,
        "expB4": expB4,
        "ident": np.eye(128, dtype=np.float32).astype(BF16),
        "patch_w": np.asarray(inputs["patch_w"]).astype(BF16),
    }
    for nm in ("wq", "wk", "wv", "wo", "w1", "w2"):
        shared[nm] = np.asarray(inputs[nm]).astype(BF16)

    # this model instance has trivial LN affine and zero residual biases
    for nm, s_, b_ in (("ln1", inputs["ln1_s"], inputs["ln1_b"]),
                       ("ln2", inputs["ln2_s"], inputs["ln2_b"]),
                       ("lnf", inputs["norm_s"], inputs["norm_b"])):
        assert np.all(np.asarray(s_) == 1.0) and np.all(np.asarray(b_) == 0.0), \
            f"{nm} affine unsupported"
    for nm in ("bo", "b1", "b2", "patch_b"):
        assert np.all(np.asarray(inputs[nm]) == 0.0), f"{nm} nonzero unsupported"

    return per_core, shared


def kernel(**inputs):
    from concourse.bass_utils import run_bass_kernel_spmd

    per_core, shared = _host_prep(inputs)
    if "prog" not in _prog_cache:
        _prog_cache["prog"] = _build_program()
    nc = _prog_cache["prog"]

    in_maps = []
    for c in range(NCORES):
        m = dict(shared)
        m.update(per_core[c])
        in_maps.append(m)
    import os
    trace = bool(os.environ.get("KERNEL_TRACE"))
    res = run_bass_kernel_spmd(nc, in_maps, core_ids=list(range(NCORES)),
                               trace=trace)
    global _last_exec_ns
    _last_exec_ns = res.exec_time_ns

    out = np.zeros((L, D), np.float32)
    for c in range(NCORES):
        out[1 + c * LC:1 + (c + 1) * LC] = res.results[c]["out"][0:LC]
    out[0] = res.results[0]["out"][LC]
    out[L - 1] = res.results[0]["out"][LC + 1]
    return out.reshape(1, L, D)


# revision 4
# speedup vs baseline: 1.0522x; 1.0379x over previous
"""Trainium2 Bass kernel for sparse-attention 3D-ViT (nn_BaseModel_44341242364529).

No-collective design: the layer-1 AllGather of band k/v is replaced by
redundant layer-0 compute.  Each core computes layer 0 (attention + FFN)
for its full 12-tile query band (rows c*512-1024 .. c*512+512), the two
corner tiles EOS attends (29, 31), and the BOS/EOS tile -- so layer-1
k/v for every key any own-query needs is produced locally and no core
ever waits on another (cross-core barrier waits dominated the measured
exec time of the collective version).

Mechanics (inherited from the collective baseline):
- all matmuls bf16, fp32 PSUM accumulation
- multiplicative attention bias: P = exp(scale*S) * expW, expW tables
  host-precomputed (masked slots = 0; fully-padded query rows attend BOS
  with weight 1 so their denominator stays finite); tables are STREAMED
  from DRAM into the P tile and multiplied in place (16MB total, too big
  to keep resident)
- attention runs in 4-query-tile "group passes" over a 12-key-tile
  window (the RUNS table): layer 0 = 3 band passes, layer 1 = 1 pass;
  corner/BOS-EOS queries use small dense per-tile passes
- w-grouped S matmuls (stationary kT tile), transposed AV (stationary
  65-col v tile with ones column for the denominator), per-head wo with
  the softmax denominator folded into the residual update
- LN per-tile (bn_stats -> sqrt+recip -> gpsimd apply -> DMA transposes)
"""

import numpy as np
import ml_dtypes

# model dims (hardcoded per spec)
IMG, PATCH, D, H, NLAYERS, DFF = 64, 4, 256, 4, 2, 1024
GT = IMG // PATCH          # 16
N = GT * GT * GT           # 4096
L = N + 2                  # 4098
DH = D // H                # 64
PVOL = PATCH ** 3          # 64
NCORES = 8
LC = 512                   # real patch rows per core
LLOC = 640                 # padded local rows (5 tiles of 128)
SCALE = 1.0 / np.sqrt(DH)  # 0.125
BF16 = ml_dtypes.bfloat16

NBK = 20                   # band key tiles (global tiles 4c-16 .. 4c+3)
NBQ = 12                   # band query tiles (= band key tiles 8..19)
CKT = [21, 23, 24, 25, 26, 27, 28, 29, 30, 31]   # corner key tiles (global)
NCK = len(CKT)
# corner query tiles 29, 31: key lists in corner-array indices (+ BOS slot)
KTS29 = [0, 2, 3, 4, 5, 6, 7]
KTS31 = [1, 4, 5, 6, 7, 8, 9]
NQT = 15                   # x_q slots: 12 band + 2 corner + 1 BOS/EOS

# w-grouped attention runs for a 4-query-tile group over a 12-tile key
# window: (w, tlist, av_start, av_stop).  w numeric = window tile, "loc4"
# = the local BOS/EOS tile (BOS key at partition 0).
RUNS = [("loc4", (0, 1, 2, 3), True, False),
        (8, (0, 1, 2, 3), False, False), (9, (1, 2, 3), False, False),
        (10, (2, 3), False, False), (11, (3,), False, False),
        (0, (0,), False, False), (1, (1,), False, False), (2, (2,), False, False),
        (3, (0,), False, False), (3, (3,), False, False),
        (4, (0, 1), False, False), (5, (0, 1, 2), False, False),
        (6, (0, 1, 2, 3), False, False), (7, (0, 1, 2, 3), False, True)]
RUNCOLS = []
_off = 0
for _w, _ts, _a, _b in RUNS:
    RUNCOLS.append(_off)
    _off += len(_ts) * 128
EXPW_COLS = _off  # 4096

# run groups for coarse exp/mult: contiguous run ranges, <=1024 cols each
GROUPS = []
_g0 = 0
for _ri in range(len(RUNS) + 1):
    if (_ri == len(RUNS) or _ri == 5
            or (RUNCOLS[_ri] - RUNCOLS[_g0]) + len(RUNS[_ri][1]) * 128 > 1024):
        GROUPS.append((_g0, _ri, RUNCOLS[_g0],
                       (RUNCOLS[_ri - 1] + len(RUNS[_ri - 1][1]) * 128) - RUNCOLS[_g0]))
        _g0 = _ri
        if _ri == len(RUNS):
            break


_prog_cache = {}


def _build_program():
    import concourse.bass as bass
    import concourse.bacc as bacc
    import concourse.tile as tile
    from concourse import mybir

    f32 = mybir.dt.float32
    bf16 = mybir.dt.bfloat16
    AF = mybir.ActivationFunctionType
    nc = bacc.Bacc("TRN2", target_bir_lowering=False, debug=False,
                   num_devices=NCORES)

    def din(name, shape, dt_=bf16):
        return nc.declare_dram_parameter(name, list(shape), dt_, isOutput=False)

    imgT_d = din("imgT", [PVOL, NBK * 128])
    imgTc_d = din("imgTc", [PVOL, NCK * 128])
    emb4_d = din("emb4", [128, D], f32)
    pw_d = din("patch_w", [PVOL, D])
    wq_d = din("wq", [NLAYERS, D, D])
    wk_d = din("wk", [NLAYERS, D, D])
    wv_d = din("wv", [NLAYERS, D, D])
    wo_d = din("wo", [NLAYERS, D, D])
    w1_d = din("w1", [NLAYERS, D, DFF])
    w2_d = din("w2", [NLAYERS, DFF, D])
    expA_d = din("expA", [H, 128, EXPW_COLS])   # band pass A (queries 0..3)
    expB_d = din("expB", [H, 128, EXPW_COLS])   # band pass B (queries 4..7)
    expC_d = din("expC", [H, 128, EXPW_COLS])   # band pass C / layer-1 pass
    expCr_d = din("expCr", [2, H, 128, 8 * 128])  # corner queries 29, 31
    expB4_d = din("expB4", [H, 128, 3 * 128])     # BOS/EOS/pad queries
    ident_d = din("ident", [128, 128])            # PE-transpose identity
    out_d = nc.declare_dram_parameter("out", [LLOC, D], f32, isOutput=True)

    from contextlib import ExitStack
    with tile.TileContext(nc) as tc, ExitStack() as ctx:
        sing = ctx.enter_context(tc.tile_pool(name="sing", bufs=1))
        wk_pool = ctx.enter_context(tc.tile_pool(name="wrk", bufs=1))
        wk2_pool = ctx.enter_context(tc.tile_pool(name="wrk2", bufs=2))
        tmp_pool = ctx.enter_context(tc.tile_pool(name="tmp", bufs=8))
        pe_pool = ctx.enter_context(tc.tile_pool(name="pexp", bufs=4))
        pm_pool = ctx.enter_context(tc.tile_pool(name="pmul", bufs=4))
        ps_s = ctx.enter_context(tc.tile_pool(name="pss", bufs=2, space="PSUM"))
        ps_aoT = ctx.enter_context(tc.tile_pool(name="pst", bufs=2, space="PSUM"))
        ps_pr = ctx.enter_context(tc.tile_pool(name="psp", bufs=2, space="PSUM"))

        sync = nc.sync

        # ---------------- constants / weights ----------------
        # DMA queue drains serially: request in first-use order
        pw = sing.tile([PVOL, D], bf16, tag="pw")
        sync.dma_start(out=pw[:], in_=pw_d[:, :])
        ident = sing.tile([128, 128], bf16, tag="ident")
        sync.dma_start(out=ident[:], in_=ident_d[:, :])
        imgT = sing.tile([PVOL, NBK * 128], bf16, tag="imgT")
        sync.dma_start(out=imgT[:], in_=imgT_d[:, :])
        imgTc = sing.tile([PVOL, NCK * 128], bf16, tag="imgTc")
        sync.dma_start(out=imgTc[:], in_=imgTc_d[:, :])
        emb4 = sing.tile([128, D], f32, tag="emb4")
        sync.dma_start(out=emb4[:], in_=emb4_d[:, :])

        W = {}

        def loadw(nm, dt_, l):
            kd = 8 if nm == "w2" else 2
            t_ = sing.tile([128, kd, dt_.shape[2]], bf16, tag=f"{nm}{l}")
            sync.dma_start(out=t_[:], in_=dt_[l].rearrange("(k p) n -> p k n", p=128))
            W[(nm, l)] = t_

        def loadwoh(l):
            t_ = sing.tile([64, H, D], bf16, tag=f"woh{l}")
            sync.dma_start(out=t_[:], in_=wo_d[l].rearrange("(h p) n -> p h n", p=64))
            W[("woh", l)] = t_

        for nm, dt_ in (("wk", wk_d), ("wv", wv_d), ("wq", wq_d)):
            loadw(nm, dt_, 0)
        loadwoh(0)
        loadw("wo", wo_d, 0)

        def load_rest():
            for nm, dt_ in (("w1", w1_d), ("w2", w2_d)):
                loadw(nm, dt_, 0)
            for nm, dt_ in (("wk", wk_d), ("wv", wv_d), ("wq", wq_d),
                            ("wo", wo_d), ("w1", w1_d), ("w2", w2_d)):
                loadw(nm, dt_, 1)
            loadwoh(1)

        eps_sb = sing.tile([128, 1], f32, tag="eps")
        nc.vector.memset(eps_sb[:], 1e-5)

        # ---------------- persistent activations ----------------
        x_q = wk_pool.tile([128, NQT, D], f32, tag="xq")
        hT = wk_pool.tile([128, 2, 31 * 128], bf16, tag="hT")
        qT0 = wk_pool.tile([128, 2, NQT * 128], bf16, tag="qT0")
        qT1 = wk_pool.tile([128, 2, 5 * 128], bf16, tag="qT1")
        kT_band = wk_pool.tile([128, 2, NBK * 128], bf16, tag="kband")
        v_band = wk_pool.tile([128, NBK, H, DH + 1], bf16, tag="vband")
        kT_cor = wk_pool.tile([128, 2, NCK * 128], bf16, tag="kcor")
        v_cor = wk_pool.tile([128, NCK, H, DH + 1], bf16, tag="vcor")
        kT4 = wk_pool.tile([128, 2, 128], bf16, tag="kT4")
        v4 = wk_pool.tile([128, H, DH + 1], bf16, tag="v4")
        kT1b = wk_pool.tile([128, 2, NBQ * 128], bf16, tag="k1b")
        v1b = wk_pool.tile([128, NBQ, H, DH + 1], bf16, tag="v1b")
        kT1x = wk_pool.tile([128, 2, 256], bf16, tag="k1x")
        v1x = wk_pool.tile([128, 2, H, DH + 1], bf16, tag="v1x")
        kT41 = wk_pool.tile([128, 2, 128], bf16, tag="kT41")
        v41 = wk_pool.tile([128, H, DH + 1], bf16, tag="v41")
        hT2 = wk_pool.tile([128, 2, NQT * 128], bf16, tag="hT2")
        yT = wk_pool.tile([128, 8, 5 * 128], bf16, tag="yT")
        stats = wk_pool.tile([128, 32, 6], f32, tag="stats")
        mv = wk_pool.tile([128, 32, 2], f32, tag="mv")
        rstd = wk_pool.tile([128, 32], f32, tag="rstd")
        o_sb = wk_pool.tile([128, 5, D], f32, tag="osb")
        aoT_sb = wk_pool.tile([128, H, 512], bf16, tag="aoTsb")

        # ones columns for the denominator trick
        nc.vector.memset(v_band[:, :, :, DH:DH + 1], 1.0)
        nc.vector.memset(v_cor[:, :, :, DH:DH + 1], 1.0)
        nc.vector.memset(v4[:, :, DH:DH + 1], 1.0)
        nc.vector.memset(v1b[:, :, :, DH:DH + 1], 1.0)
        nc.vector.memset(v1x[:, :, :, DH:DH + 1], 1.0)
        nc.vector.memset(v41[:, :, DH:DH + 1], 1.0)

        # ---------------- helpers ----------------
        _ln_i = [0]

        def ln_batch(pairs):
            """pairs: [(src f32 [128,D], dst bf16/f32 [128,D])], stage-batched
            (all stats, one sqrt, one recip, all applies) so each engine's
            FIFO streams independent work instead of per-tile chains."""
            n = len(pairs)
            assert n <= 16
            i0 = 0 if _ln_i[0] % 32 + n > 32 else _ln_i[0] % 32
            _ln_i[0] = i0 + n
            for i, (src, dst) in enumerate(pairs):
                nc.vector.bn_stats(out=stats[:, i0 + i, :], in_=src)
                nc.vector.bn_aggr(out=mv[:, i0 + i, :], in_=stats[:, i0 + i, :])
            nc.scalar.activation(out=rstd[:, i0:i0 + n],
                                 in_=mv[:, i0:i0 + n, 1],
                                 func=AF.Sqrt, bias=eps_sb[:], scale=1.0)
            nc.vector.reciprocal(out=rstd[:, i0:i0 + n], in_=rstd[:, i0:i0 + n])
            for i, (src, dst) in enumerate(pairs):
                nc.gpsimd.tensor_scalar(out=dst, in0=src,
                                        scalar1=mv[:, i0 + i, 0:1],
                                        scalar2=rstd[:, i0 + i:i0 + i + 1],
                                        op0=mybir.AluOpType.subtract,
                                        op1=mybir.AluOpType.mult)

        def pe_transpose(srcs, dst, slot0):
            """PE-transpose <=2 row-major bf16 [128,D] tiles into feature-major
            hT slots (keeps the DMA queue free; bf16 PSUM pass-through)."""
            n = len(srcs)
            ps = ps_aoT.tile([128, 1024], bf16, tag="tr", bufs=1)
            with nc.allow_low_precision(reason="transpose is pass-through"):
                for i, src in enumerate(srcs):
                    for j in range(2):
                        nc.tensor.matmul(
                            ps[:, (2 * i + j) * 128:(2 * i + j + 1) * 128],
                            lhsT=src[:, j * 128:(j + 1) * 128], rhs=ident[:],
                            is_transpose=True, start=True, stop=True)
            nc.vector.tensor_copy(
                out=dst[:, :, slot0 * 128:(slot0 + n) * 128].rearrange(
                    "p j (t c) -> p j t c", t=n),
                in_=ps[:, 0:n * 256].rearrange("p (t j c) -> p j t c",
                                               j=2, c=128))

        def k_proj_batch(wsb, hslot0, n, kdst, kcol0):
            """k for n (<=4) consecutive hT slots in one psum tile per
            j-block: 4x fewer DVE copies (fixed op overhead dominates)."""
            for j in range(2):
                ps = ps_pr.tile([128, 512], f32, tag="pr")
                for i in range(2):
                    nc.tensor.matmul(
                        ps[:, 0:n * 128],
                        lhsT=wsb[:, i, j * 128:(j + 1) * 128],
                        rhs=hT[:, i, hslot0 * 128:(hslot0 + n) * 128],
                        start=(i == 0), stop=(i == 1))
                nc.vector.tensor_copy(out=kdst[:, j, kcol0:kcol0 + n * 128],
                                      in_=ps[:, 0:n * 128])

        def k_proj(wsb, hslot, kdst, kcols):
            """kdst[:, j, kcols:+128] = wk_j^T @ h  for j in 0..1"""
            for j in range(2):
                ps = ps_pr.tile([128, 512], f32, tag="pr")
                for i in range(2):
                    nc.tensor.matmul(ps[:, 0:128],
                                     lhsT=wsb[:, i, j * 128:(j + 1) * 128],
                                     rhs=hT[:, i, hslot * 128:hslot * 128 + 128],
                                     start=(i == 0), stop=(i == 1))
                nc.vector.tensor_copy(out=kdst[:, j, kcols:kcols + 128],
                                      in_=ps[:, 0:128])

        def v_proj(wsb, hslot, dst_hx):
            ps = ps_aoT.tile([128, 512], f32, tag="aoT", bufs=1)
            for i in range(2):
                nc.tensor.matmul(ps[:, 0:D],
                                 lhsT=hT[:, i, hslot * 128:hslot * 128 + 128],
                                 rhs=wsb[:, i, :],
                                 start=(i == 0), stop=(i == 1))
            nc.vector.tensor_copy(out=dst_hx,
                                  in_=ps[:, 0:D].rearrange("p (h x) -> p h x",
                                                           h=H))

        def q_proj(wsb, hslot, qdst, qcols):
            ps = ps_pr.tile([128, 512], f32, tag="pr")
            for j in range(2):
                for i in range(2):
                    nc.tensor.matmul(ps[:, j * 128:(j + 1) * 128],
                                     lhsT=wsb[:, i, j * 128:(j + 1) * 128],
                                     rhs=hT[:, i, hslot * 128:hslot * 128 + 128],
                                     start=(i == 0), stop=(i == 1))
            nc.vector.tensor_copy(
                out=qdst[:, :, qcols:qcols + 128],
                in_=ps[:, 0:D].rearrange("p (j c) -> p j c", j=2))

        # ---- group pass: 4 query tiles x 12-key-tile window ----
        def attn_pass(kTw, vw, g, kT4_l, v4_l, qT_l, qoff, expD, woh, xslots):
            """One 4-query-tile attention pass.
            kTw/vw: key window tensors; g: window tile offset; qT_l/qoff:
            query tensor + col offset; expD: DRAM exp table [H,128,4096];
            xslots: x_q slot per local query tile (residual target)."""
            phs = []
            for hh in range(H):
                pb, dt_ = (hh % 2) * 64, hh // 2
                ph = pe_pool.tile([128, EXPW_COLS], bf16, tag="ph")
                phs.append(ph)
                nc.gpsimd.dma_start(out=ph[:, :], in_=expD[hh, :, :])
                for g0, g1, goff, gcols in GROUPS:
                    st = ps_s.tile([128, 1024], f32, tag="st")
                    for ri in range(g0, g1):
                        w, ts, _a, _b = RUNS[ri]
                        ncol = len(ts) * 128
                        q0 = qoff + ts[0] * 128
                        lo = RUNCOLS[ri] - goff
                        if w == "loc4":
                            kl = kT4_l[pb:pb + 64, dt_, :]
                        else:
                            kl = kTw[pb:pb + 64, dt_,
                                     (g + w) * 128:(g + w + 1) * 128]
                        cuts = [0, ncol]
                        if lo < 512 < lo + ncol:
                            cuts = [0, 512 - lo, ncol]
                        for a, b in zip(cuts[:-1], cuts[1:]):
                            nc.tensor.matmul(
                                st[:, lo + a:lo + b], lhsT=kl,
                                rhs=qT_l[pb:pb + 64, dt_, q0 + a:q0 + b],
                                start=True, stop=True)
                    pe = pm_pool.tile([128, 1024], bf16, tag="pe", bufs=3)
                    nc.scalar.activation(out=pe[:, 0:gcols], in_=st[:, 0:gcols],
                                         func=AF.Exp, scale=float(SCALE))
                    nc.vector.tensor_mul(ph[:, goff:goff + gcols],
                                         pe[:, 0:gcols],
                                         ph[:, goff:goff + gcols])
            for hh in range(H):
                ph = phs[hh]
                aoTp = ps_aoT.tile([128, 512], f32, tag="aoT", bufs=1)
                for ri, (w, ts, av_s, av_e) in enumerate(RUNS):
                    ncol = len(ts) * 128
                    q0 = ts[0] * 128
                    if w == "loc4":
                        vv = v4_l[:, hh, :]
                    else:
                        vv = vw[:, g + w, hh, :]
                    nc.tensor.matmul(
                        aoTp[0:65, q0:q0 + ncol], lhsT=vv,
                        rhs=ph[:, RUNCOLS[ri]:RUNCOLS[ri] + ncol],
                        start=av_s, stop=av_e)
                nc.vector.tensor_copy(out=aoT_sb[0:65, hh, :], in_=aoTp[0:65, :])
                # fold 1/den into aoT along the query (free) axis: recip the
                # den row down to partition 0 (partition_broadcast reads
                # physical partition 0), broadcast across dh partitions, mult
                dr = wk2_pool.tile([1, 512], bf16, tag="dr", bufs=1)
                with nc.allow_low_precision(reason="bf16 den recip, 2e-2 gate"):
                    nc.vector.reciprocal(out=dr[0:1, :],
                                         in_=aoT_sb[64:65, hh, :])
                dbc = wk2_pool.tile([64, 512], bf16, tag="dbc")
                nc.gpsimd.partition_broadcast(out_ap=dbc[:], in_ap=dr[0:1, :],
                                              channels=64)
                nc.vector.tensor_mul(aoT_sb[0:64, hh, :],
                                     aoT_sb[0:64, hh, :], dbc[:])
            for t in range(4):
                xo = ps_pr.tile([128, 512], f32, tag="pr")
                for hh in range(H):
                    nc.tensor.matmul(xo[:, 0:D],
                                     lhsT=aoT_sb[0:64, hh, t * 128:(t + 1) * 128],
                                     rhs=woh[:, hh, :], start=(hh == 0),
                                     stop=(hh == H - 1))
                nc.vector.tensor_add(x_q[:, xslots[t], :],
                                     x_q[:, xslots[t], :], xo[:, 0:D])

        # ---- dense pass: one query tile, explicit key-tile list ----
        def dense_pass(kts, kTx_l, vx_l, kT4_l, v4_l, qT_l, qcol, expD,
                       wo_sb, xslot):
            """kts: list of ("c", idx) / ("loc4",); expD [H, 128, nkt*128]."""
            nkt = len(kts)
            nc_cols = nkt * 128
            ao_ps = ps_pr.tile([128, 512], f32, tag="pr")
            for hh in range(H):
                pb, dt_ = (hh % 2) * 64, hh // 2
                ph = pm_pool.tile([128, 1024], bf16, tag="pe", bufs=3)
                nc.gpsimd.dma_start(out=ph[:, 0:nc_cols],
                                    in_=expD[hh, :, 0:nc_cols])
                st = ps_s.tile([128, 1024], f32, tag="st")
                for ki, kt in enumerate(kts):
                    if kt[0] == "loc4":
                        lhsT = kT4_l[pb:pb + 64, dt_, :]
                    else:
                        w = kt[1]
                        lhsT = kTx_l[pb:pb + 64, dt_, w * 128:(w + 1) * 128]
                    nc.tensor.matmul(st[:, ki * 128:(ki + 1) * 128],
                                     lhsT=lhsT,
                                     rhs=qT_l[pb:pb + 64, dt_, qcol:qcol + 128],
                                     start=True, stop=True)
                pe = pm_pool.tile([128, 1024], bf16, tag="pe", bufs=3)
                nc.scalar.activation(out=pe[:, 0:nc_cols], in_=st[:, 0:nc_cols],
                                     func=AF.Exp, scale=float(SCALE))
                nc.vector.tensor_mul(ph[:, 0:nc_cols], pe[:, 0:nc_cols],
                                     ph[:, 0:nc_cols])
                for ki, kt in enumerate(kts):
                    if kt[0] == "loc4":
                        rhs = v4_l[:, hh, :]
                    else:
                        rhs = vx_l[:, kt[1], hh, :]
                    nc.tensor.matmul(ao_ps[:, hh * 65:hh * 65 + 65],
                                     lhsT=ph[:, ki * 128:(ki + 1) * 128],
                                     rhs=rhs, start=(ki == 0),
                                     stop=(ki == nkt - 1))
            rec = wk2_pool.tile([128, 4], f32, tag="rec")
            nc.vector.reciprocal(out=rec[:], in_=ao_ps[:, DH:260:DH + 1])
            ao_sb = wk2_pool.tile([128, D], bf16, tag="aosb")
            for hh in range(H):
                nc.vector.tensor_scalar(
                    out=ao_sb[:, hh * DH:(hh + 1) * DH],
                    in0=ao_ps[:, hh * 65:hh * 65 + DH],
                    scalar1=rec[:, hh:hh + 1], scalar2=None,
                    op0=mybir.AluOpType.mult)
            aoT = wk2_pool.tile([128, 2, 128], bf16, tag="aoT")
            pe_transpose([ao_sb[:]], aoT, 0)
            xo = ps_pr.tile([128, 512], f32, tag="pr")
            for i in range(2):
                nc.tensor.matmul(xo[:, 0:D], lhsT=aoT[:, i, :],
                                 rhs=wo_sb[:, i, :],
                                 start=(i == 0), stop=(i == 1))
            nc.vector.tensor_add(x_q[:, xslot, :], x_q[:, xslot, :], xo[:, 0:D])

        # ---- FFN over x_q slots: all LN+transposes first (one sqrt/gelu
        # table transition), then 5-tile gelu/w2 waves ----
        def ffn(slots, w1sb, w2sb):
            hs = []
            for group in range(0, len(slots), 8):
                gs = slots[group:group + 8]
                ghs = []
                for s in gs:
                    h2 = tmp_pool.tile([128, D], bf16, tag="h2", bufs=4)
                    ghs.append(h2)
                ln_batch([(x_q[:, s, :], ghs[i][:]) for i, s in enumerate(gs)])
                for i in range(0, len(gs), 2):
                    pe_transpose([h[:] for h in ghs[i:i + 2]], hT2,
                                 group + i)
                hs += ghs
            for w0 in range(0, len(slots), 5):
                wslots = slots[w0:w0 + 5]
                n = len(wslots)
                for fj in range(8):
                    c0 = 0
                    while c0 < n * 128:
                        c1 = min(c0 + 512, n * 128)
                        ps = ps_pr.tile([128, 512], f32, tag="pr")
                        for i in range(2):
                            nc.tensor.matmul(
                                ps[:, 0:c1 - c0],
                                lhsT=w1sb[:, i, fj * 128:(fj + 1) * 128],
                                rhs=hT2[:, i, w0 * 128 + c0:w0 * 128 + c1],
                                start=(i == 0), stop=(i == 1))
                        nc.scalar.activation(out=yT[:, fj, c0:c1],
                                             in_=ps[:, 0:c1 - c0],
                                             func=AF.Gelu, scale=1.0)
                        c0 = c1
                for li, s in enumerate(wslots):
                    ps2 = ps_s.tile([128, 1024], f32, tag="st")
                    for fj in range(8):
                        nc.tensor.matmul(ps2[:, 0:D],
                                         lhsT=yT[:, fj, li * 128:(li + 1) * 128],
                                         rhs=w2sb[:, fj, :],
                                         start=(fj == 0), stop=(fj == 7))
                    nc.vector.tensor_add(x_q[:, s, :], x_q[:, s, :],
                                         ps2[:, 0:D])

        # ================= layer 0 =================
        wq0, wk0, wv0 = W[("wq", 0)], W[("wk", 0)], W[("wv", 0)]

        # tile specs: (imgT tensor | None=BOS/EOS, col, hslot, xslot, kdst,
        # kcol, vdst, qcol).  hslots are consecutive 0..30 so PE transposes
        # pack in pairs; processed in waves of 8 with stage-batched LN.
        specs = []
        for i in range(NBK):
            specs.append((imgT, i, i, i - 8 if i >= 8 else None,
                          kT_band, i * 128, v_band[:, i, :, 0:DH],
                          (i - 8) * 128 if i >= 8 else None))
        for j in range(NCK):
            xslot = 12 if j == 7 else (13 if j == 9 else None)
            specs.append((imgTc, j, 20 + j, xslot, kT_cor, j * 128,
                          v_cor[:, j, :, 0:DH],
                          xslot * 128 if xslot is not None else None))
        specs.append((None, 0, 30, 14, kT4, 0, v4[:, :, 0:DH], 14 * 128))

        for w0 in range(0, len(specs), 8):
            wave = specs[w0:w0 + 8]
            xs = []
            for (srcT, col, hslot, xslot, kdst, kcol, vdst, qcol) in wave:
                if srcT is None:
                    nc.vector.tensor_copy(out=x_q[:, 14, :], in_=emb4[:])
                    xs.append(x_q[:, 14, :])
                    continue
                ps = ps_pr.tile([128, 512], f32, tag="pr")
                nc.tensor.matmul(ps[:, 0:D],
                                 lhsT=srcT[:, col * 128:(col + 1) * 128],
                                 rhs=pw[:], start=True, stop=True)
                if xslot is not None:
                    nc.scalar.copy(out=x_q[:, xslot, :], in_=ps[:, 0:D])
                    xs.append(x_q[:, xslot, :])
                else:
                    xt = tmp_pool.tile([128, D], f32, tag="xt")
                    nc.scalar.copy(out=xt[:], in_=ps[:, 0:D])
                    xs.append(xt[:])
            hs = []
            for _ in wave:
                hs.append(tmp_pool.tile([128, D], bf16, tag="h", name="hwv", bufs=4))
            ln_batch([(xs[i], hs[i][:]) for i in range(len(wave))])
            for i in range(0, len(wave), 2):
                pe_transpose([h[:] for h in hs[i:i + 2]], hT, w0 + i)
            # batched k over consecutive same-destination runs
            runs_k = []
            for (srcT, col, hslot, xslot, kdst, kcol, vdst, qcol) in wave:
                if (runs_k and runs_k[-1][0] is kdst
                        and kcol == runs_k[-1][2] + runs_k[-1][3] * 128
                        and runs_k[-1][3] < 4):
                    runs_k[-1][3] += 1
                else:
                    runs_k.append([kdst, hslot, kcol, 1])
            for kdst_t, hslot0, kcol0, n in runs_k:
                k_proj_batch(wk0, hslot0, n, kdst_t, kcol0)
            for wi, (srcT, col, hslot, xslot, kdst, kcol, vdst, qcol) \
                    in enumerate(wave):
                v_proj(wv0, hslot, vdst)
                if qcol is not None:
                    q_proj(wq0, hslot, qT0, qcol)

        # band group passes
        woh0, wo0 = W[("woh", 0)], W[("wo", 0)]
        attn_pass(kT_band, v_band, 0, kT4, v4, qT0, 0, expA_d, woh0,
                  (0, 1, 2, 3))
        load_rest()   # FFN + layer-1 weights stream in behind pass A
        attn_pass(kT_band, v_band, 4, kT4, v4, qT0, 4 * 128, expB_d, woh0,
                  (4, 5, 6, 7))
        attn_pass(kT_band, v_band, 8, kT4, v4, qT0, 8 * 128, expC_d, woh0,
                  (8, 9, 10, 11))
        # corner + BOS/EOS dense passes
        dense_pass([("c", w) for w in KTS29] + [("loc4",)],
                   kT_cor, v_cor, kT4, v4, qT0, 12 * 128, expCr_d[0],
                   wo0, 12)
        dense_pass([("c", w) for w in KTS31] + [("loc4",)],
                   kT_cor, v_cor, kT4, v4, qT0, 13 * 128, expCr_d[1],
                   wo0, 13)
        dense_pass([("loc4",), ("c", 7), ("c", 9)],
                   kT_cor, v_cor, kT4, v4, qT0, 14 * 128, expB4_d,
                   wo0, 14)
        w10, w20 = W[("w1", 0)], W[("w2", 0)]
        ffn(list(range(NQT)), w10, w20)

        # ================= layer 1 =================
        wq1, wk1, wv1 = W[("wq", 1)], W[("wk", 1)], W[("wv", 1)]
        for group in (list(range(8)), list(range(8, NQT))):
            ghs = []
            for s in group:
                ghs.append(tmp_pool.tile([128, D], bf16, tag="h", name="hwv", bufs=4))
            ln_batch([(x_q[:, s, :], ghs[i][:]) for i, s in enumerate(group)])
            for i in range(0, len(group), 2):
                pe_transpose([h[:] for h in ghs[i:i + 2]], hT, group[i])
        for s0 in range(0, NBQ, 4):
            k_proj_batch(wk1, s0, 4, kT1b, s0 * 128)
        k_proj_batch(wk1, 12, 2, kT1x, 0)
        k_proj_batch(wk1, 14, 1, kT41, 0)
        for s in range(NBQ):
            v_proj(wv1, s, v1b[:, s, :, 0:DH])
        for xi, s in enumerate((12, 13)):
            v_proj(wv1, s, v1x[:, xi, :, 0:DH])
        v_proj(wv1, 14, v41[:, :, 0:DH])
        for t in range(4):
            q_proj(wq1, 8 + t, qT1, t * 128)
        q_proj(wq1, 14, qT1, 4 * 128)

        woh1, wo1 = W[("woh", 1)], W[("wo", 1)]
        attn_pass(kT1b, v1b, 0, kT41, v41, qT1, 0, expC_d, woh1,
                  (8, 9, 10, 11))
        dense_pass([("loc4",), ("c", 0), ("c", 1)],
                   kT1x, v1x, kT41, v41, qT1, 4 * 128, expB4_d,
                   wo1, 14)

        w11, w21 = W[("w1", 1)], W[("w2", 1)]
        ffn([8, 9, 10, 11, 14], w11, w21)

        # ---------------- final LN + output ----------------
        ln_batch([(x_q[:, s, :], o_sb[:, lt, :])
                  for lt, s in enumerate((8, 9, 10, 11, 14))])
        for lt in range(5):
            sync.dma_start(out=out_d[lt * 128:(lt + 1) * 128, :],
                           in_=o_sb[:, lt, :])

    nc.finalize()
    return nc


# ======================= host side =======================

def _patchify(img):
    x = img.reshape(1, 1, GT, PATCH, GT, PATCH, GT, PATCH)
    x = np.einsum("nctphqwr->nthwpqrc", x).reshape(N, PVOL)
    return np.ascontiguousarray(x).astype(np.float32)


def _expA_to_runs(expA):
    """regroup per-(t,ki) blocks [4,H,128,8*128] into RUNS layout [H,128,4096]"""
    expW = np.zeros((H, 128, EXPW_COLS), np.float32)
    for ri, (w, ts, _a, _b) in enumerate(RUNS):
        co = RUNCOLS[ri]
        for t in ts:
            ki = 7 if w == "loc4" else [t, t + 3, t + 4, t + 5, t + 6,
                                        t + 7, t + 8].index(w)
            expW[:, :, co + (t - ts[0]) * 128:co + (t - ts[0] + 1) * 128] = \
                expA[t, :, :, ki * 128:(ki + 1) * 128]
    return expW


def _host_prep(inputs):
    idx = np.asarray(inputs["idx"])
    valid = np.asarray(inputs["valid"])
    geo = np.asarray(inputs["geo_dist"]).astype(np.float32)
    decay = np.asarray(inputs["decay"]).astype(np.float32)
    K = idx.shape[1]
    fv = valid & (idx <= np.arange(L)[:, None])
    bias_lk = geo[None] * decay[:, None, None]          # [H, L, K]

    patches = _patchify(np.asarray(inputs["input_image"]))
    ids = np.asarray(inputs["input_ids"]).reshape(-1)
    et = np.asarray(inputs["embed_tokens"])
    bos_e, eos_e = et[ids[0]], et[ids[-1]]

    emb4 = np.zeros((128, D), np.float32)
    emb4[0] = bos_e
    emb4[1] = eos_e

    # group-pass exp tables: ct(vc) = 4-query-tile table for queries =
    # global patch tiles 4vc..4vc+3 over window tiles 4vc-8..4vc+3
    def build_ct(vc):
        expA = np.zeros((4, H, 128, 8 * 128), np.float32)
        if vc < 0:
            # fully-padded queries attend BOS with weight 1 (finite den)
            expA[:, :, 0, 7 * 128:8 * 128] = 1.0
            return _expA_to_runs(expA)
        base = vc * 512 - 1024
        for lq in range(512):
            gq = 1 + vc * 512 + lq
            t, lcol = lq // 128, lq % 128
            m = fv[gq]
            kr = idx[gq][m].astype(np.int64)
            ev = np.exp(bias_lk[:, gq, m])               # [H, nk]
            bos = kr == 0
            if bos.any():
                expA[t, :, 0, 7 * 128 + lcol] = ev[:, bos][:, 0]
            nb = ~bos
            krn = kr[nb] - 1 - base
            assert np.all((krn >= 0) & (krn < 1536)), (vc, gq)
            w, j = krn // 128, krn % 128
            off = w - t
            ki = np.where(off == 0, 0, off - 2)
            assert np.all(((off == 0) | ((off >= 3) & (off <= 8)))), (vc, gq)
            expA[t, :, j, ki * 128 + lcol] = ev[:, nb].T
        return _expA_to_runs(expA)

    ct = {-2: build_ct(-2).astype(BF16)}   # pad table (all queries padded)
    ct[-1] = ct[-2]
    for vc in range(NCORES):
        ct[vc] = build_ct(vc).astype(BF16)

    # corner-query tables (tiles 29, 31) -- shared across cores
    g2a = {g: a for a, g in enumerate(CKT)}
    expCr = np.zeros((2, H, 128, 8 * 128), np.float32)
    for ti, T in enumerate((29, 31)):
        kts_g = [T - 8, T - 5, T - 4, T - 3, T - 2, T - 1, T]
        for lcol in range(128):
            gq = 1 + T * 128 + lcol
            m = fv[gq]
            kr = idx[gq][m].astype(np.int64)
            ev = np.exp(bias_lk[:, gq, m])
            bos = kr == 0
            if bos.any():
                expCr[ti, :, 0, 7 * 128 + lcol] = ev[:, bos][:, 0]
            nb = ~bos
            krn = kr[nb] - 1
            KT, j = krn // 128, krn % 128
            ki = np.array([kts_g.index(int(x)) for x in KT])
            expCr[ti, :, j, ki * 128 + lcol] = ev[:, nb].T
    expCr = expCr.astype(BF16)

    # BOS/EOS/pad-query table (kts = [loc4, corner7(=29), corner9(=31)])
    expB4 = np.zeros((H, 128, 3 * 128), np.float32)
    expB4[:, 0, 2:128] = 1.0                             # pad queries attend BOS
    for li, gq in ((0, 0), (1, L - 1)):
        for k in range(K):
            if not fv[gq, k]:
                continue
            kr = int(idx[gq, k])
            ev = np.exp(bias_lk[:, gq, k])
            if kr == 0:
                expB4[:, 0, li] = ev
            elif kr == L - 1:
                expB4[:, 1, li] = ev
            else:
                p = kr - 1
                if 3712 <= p < 3840:
                    expB4[:, p - 3712, 1 * 128 + li] = ev
                elif 3968 <= p < 4096:
                    expB4[:, p - 3968, 2 * 128 + li] = ev
                else:
                    raise AssertionError((gq, kr))
    expB4 = expB4.astype(BF16)

    # corner-key image tiles (shared)
    imgTc = np.concatenate([patches[T * 128:(T + 1) * 128].T for T in CKT],
                           axis=1).astype(BF16)          # [64, 1280]

    per_core = []
    for c in range(NCORES):
        imgT = np.zeros((PVOL, NBK * 128), np.float32)
        lo = c * LC - 2048
        s0, s1 = max(0, -lo), min(NBK * 128, N - lo)
        imgT[:, s0:s1] = patches[lo + s0:lo + s1].T
        per_core.append({"imgT": imgT.astype(BF16),
                         "expA": np.ascontiguousarray(ct[c - 2]),
                         "expB": np.ascontiguousarray(ct[c - 1]),
                         "expC": np.ascontiguousarray(ct[c])})

    shared = {
        "imgTc": imgTc,
        "emb4": emb4,
        "expCr": expCr,
        "expB4": expB4,
        "ident": np.eye(128, dtype=np.float32).astype(BF16),
        "patch_w": np.asarray(inputs["patch_w"]).astype(BF16),
    }
    for nm in ("wq", "wk", "wv", "wo", "w1", "w2"):
        shared[nm] = np.asarray(inputs[nm]).astype(BF16)

    # this model instance has trivial LN affine and zero residual biases
    for nm, s_, b_ in (("ln1", inputs["ln1_s"], inputs["ln1_b"]),
                       ("ln2", inputs["ln2_s"], inputs["ln2_b"]),
                       ("lnf", inputs["norm_s"], inputs["norm_b"])):
        assert np.all(np.asarray(s_) == 1.0) and np.all(np.asarray(b_) == 0.0), \
            f"{nm} affine unsupported"
    for nm in ("bo", "b1", "b2", "patch_b"):
        assert np.all(np.asarray(inputs[nm]) == 0.0), f"{nm} nonzero unsupported"

    return per_core, shared


def kernel(**inputs):
    from concourse.bass_utils import run_bass_kernel_spmd

    per_core, shared = _host_prep(inputs)
    if "prog" not in _prog_cache:
        _prog_cache["prog"] = _build_program()
    nc = _prog_cache["prog"]

    in_maps = []
    for c in range(NCORES):
        m = dict(shared)
        m.update(per_core[c])
        in_maps.append(m)
    import os
    trace = bool(os.environ.get("KERNEL_TRACE"))
    res = run_bass_kernel_spmd(nc, in_maps, core_ids=list(range(NCORES)),
                               trace=trace)
    global _last_exec_ns
    _last_exec_ns = res.exec_time_ns

    out = np.zeros((L, D), np.float32)
    for c in range(NCORES):
        out[1 + c * LC:1 + (c + 1) * LC] = res.results[c]["out"][0:LC]
    out[0] = res.results[0]["out"][LC]
    out[L - 1] = res.results[0]["out"][LC + 1]
    return out.reshape(1, L, D)
